# revision 23
# baseline (speedup 1.0000x reference)
"""Local (windowed) attention Trainium2 Bass kernel.

Problem: q,k,v [8, 8, 4096, 64] fp32; window 128, look_backward 1, pad -1.0.
out[b,h,w,i,:] = softmax(scale * q_wi . [k_{w-1}; k_w]) @ [v_{w-1}; v_w]
(with window -1 = all -1.0 pad values, which DO enter the softmax).

Sharding: data-parallel over flat batch*heads (64) -> 8 heads per core.

Per-core layouts (prepared host-side, all fp16 -- fp8 variants measured
over the 2e-2 error gate host-side, so inputs stay 16-bit):
  qT : [4, 128, 4096]  head pair stacked on partitions (d=64 each),
                       free axis = 4096 queries (d-major transposed)
  kT : [4, 128, 4224]  same, one pad chunk (128 keys of -1.0) prepended
                       -> 33 chunks of 128 keys
  v  : [8, 128, 33, 65] per head; partition = key-within-chunk, pad chunk
                       prepended; col 64 = 1.0 (ones col yields softmax l)
  out: [4, 128, 32, 130] UNNORMALIZED out cols for both heads of a pair
                       interleaved per window ([w, h, 65]; col 64 of each
                       head's 65 = denominator l); host divides + unpacks.

Device pipeline per head pair, per key-chunk group (2 chunks):
  MM1 (fp16): scoresT[j, i] per chunk/head; heads alternate PE row-group
              bases 0/64 (run concurrently); psum layout [h0:512|h1:512]
              so each bank sees a single weight base partition (mixing
              bases within a bank hard-crashes the device).
  exp: one full-tile op per group, psum fp32 -> 16-bit P tiles.  Split
       across Act (exact table exp) and DVE (Schraudolph exp2 writing
       bf16 bits via an int16 bitcast) to share the 1 elem/cycle/lane
       elementwise throughput wall; the split ratio is the engine
       balance knob.
  MM2 (16b): out_w[i, 0:65] += P_blockT @ v_aug[p] into merged-head psum
       accumulators [128, 3, 130] (one 2KB bank holds 3 windows x both
       heads), deferred two groups behind MM1 so exp never waits on PE.
  evac: one psum->fp16 copy per 3-window batch covering both heads
       (Act and DVE alternate by a balance set); stores stream out on
       the gpsimd queue in 3 slices per pair.

DMA: kt+qt ride the sync HWDGE queue, v + output stores the gpsimd
SWDGE queue; pair p+1's loads are prefetched during pair p's compute so
the PE/exp engines never see a pair-boundary stall (also keeps the PE's
HAM clock-gate warm).

Accuracy: ~1.1e-2 relative vs the 2e-2 gate (host-simulated and HW
measured): Schraudolph mantissa interpolation (~3%/elem, mostly
cancelled by softmax renormalization) on the DVE share; fp16 operands
contribute ~6e-4.
"""

import os
import sys

for _p in ("/opt/trn_rl_repo", "/opt/pypackages"):
    if os.path.isdir(_p) and _p not in sys.path:
        sys.path.append(_p)

import numpy as np

import concourse.mybir as mybir
import concourse.tile as tile
from concourse import bacc
from concourse.bass_utils import run_bass_kernel_spmd

B, H, N, D = 8, 8, 4096, 64
WS = 128                 # window size
W = N // WS              # 32 windows
C = W + 1                # 33 key chunks incl. pad chunk
NC = 8                   # cores
HPC = (B * H) // NC      # 8 heads per core
PAIRS = HPC // 2         # 4 head pairs per core
SCALE = float(D) ** -0.5

MM1_DT = mybir.dt.float16
MM2_DT = mybir.dt.float16
GROUP = 2                # key chunks per exp batch
EB = 3                   # windows per merged-head psum accumulator bank
                         # (3 * 130 * 4B = 1560 <= 2KB)
NB = (W + EB - 1) // EB  # 11 evac batches per pair (last has 2 windows)

# Schraudolph exp2 offload: for a subset of groups the exp runs on the DVE
# instead of the Act engine, directly in the bf16 bit domain:
#   int16 t = round(s * (128*log2e*SCALE) + (16256 - C))
# t's bits ARE bf16(2^(s*log2e*SCALE)) with linear mantissa interpolation
# (~3% deterministic error); softmax renormalization cancels most of it.
# Exp-engine assignment per pair (group indices exp'd on the DVE, rest
# on Act).  ~6 of 17 groups on DVE plus all evacs balances the two
# engines at ~12.5us per pair (the elementwise throughput wall).
# Pair 0 runs DMA-paced (inputs still streaming in), so Act takes almost
# everything there - DVE would otherwise idle during rampup anyway and
# Act's spare capacity is free.  The last pair puts the DVE groups
# early so both engines drain together at the end.
SCH_OFF_BY_PAIR = [
    {8, 14},
    {2, 5, 8, 11, 14, 16},
    {2, 5, 8, 11, 14, 16},
    {1, 4, 7, 10, 13},
]
SCH_A = float(128.0 / np.log(2.0) * SCALE)
SCH_B = float(16256.0 - 5.59)

_NC_CACHE = {}


def build_nc(pairs=PAIRS, w=W):
    c = w + 1
    n = w * WS
    nb = (w + EB - 1) // EB
    nc = bacc.Bacc("TRN2", target_bir_lowering=False)
    qT = nc.dram_tensor("qT", [pairs, 128, n], MM1_DT, kind="ExternalInput")
    kT = nc.dram_tensor("kT", [pairs, 128, c * WS], MM1_DT, kind="ExternalInput")
    vv = nc.dram_tensor("v", [2 * pairs, 128, c, D + 1], MM2_DT, kind="ExternalInput")
    out = nc.dram_tensor("out", [pairs, 128, w, 2 * (D + 1)], mybir.dt.float16,
                         kind="ExternalOutput")

    f32 = mybir.dt.float32
    Exp = mybir.ActivationFunctionType.Exp

    with tile.TileContext(nc) as tc:
        with (
            tc.tile_pool(name="qk", bufs=2) as qk_pool,
            tc.tile_pool(name="vp", bufs=4) as v_pool,
            tc.tile_pool(name="pt", bufs=5) as pt_pool,
            tc.tile_pool(name="st", bufs=2) as st_pool,
            tc.tile_pool(name="ps_s", bufs=3, space="PSUM") as ps_s,
            tc.tile_pool(name="ps_o", bufs=2, space="PSUM") as ps_o,
        ):
            # --- tiles for all pairs, loads for pair 0 up front ---------
            # stg is split into one tile per output store slice so an
            # evac never WAR-waits on a still-in-flight DMA store of an
            # earlier slice (DMA reads hold the whole tile).  The last
            # pair stores batch-by-batch (whole small tiles) so the
            # post-compute drain tail is minimal.
            SLICES_STD = [(0, 12), (12, 21), (21, 30), (30, w)]
            SLICES_LAST = [(0, 12), (12, 21), (21, 24), (24, 27), (27, 30),
                           (30, w)]
            qts, kts, vts, stgs = [], [], [], []
            for pair in range(pairs):
                qts.append(qk_pool.tile([128, n], MM1_DT, tag="qT",
                                        name=f"qt_{pair}"))
                kts.append(qk_pool.tile([128, c * WS], MM1_DT, tag="kT",
                                        name=f"kt_{pair}"))
                vts.append([v_pool.tile([128, c, D + 1], MM2_DT, tag="v",
                                        name=f"v_{pair}_{h}") for h in range(2)])
                slices = SLICES_LAST if pair == pairs - 1 else SLICES_STD
                stgs.append([st_pool.tile([128, e - s, 2 * (D + 1)],
                                          mybir.dt.float16, tag=f"stg{si}",
                                          name=f"stg_{pair}_{si}")
                             for si, (s, e) in enumerate(slices)])

            def load_pair(pair, startup=False):
                qt, kt, vt = qts[pair], kts[pair], vts[pair]
                if startup:
                    # fine-grained need-ordered first slices so MM1 starts
                    # ~1.5us in; qt rides the scalar HWDGE queue (Act is
                    # idle during startup) so kt/qt stream in parallel
                    kb = [0, 256, 1024, 2304, c * WS]
                    qb = [0, 384, 1024, 2304, n]
                    nc.sync.dma_start(kt[:, kb[0]:kb[1]], kT[pair][:, kb[0]:kb[1]])
                    nc.scalar.dma_start(qt[:, qb[0]:qb[1]], qT[pair][:, qb[0]:qb[1]])
                    nc.gpsimd.dma_start(vt[0][:, 0:5], vv[2 * pair][:, 0:5])
                    nc.gpsimd.dma_start(vt[1][:, 0:5], vv[2 * pair + 1][:, 0:5])
                    for sl in (1, 2, 3):
                        nc.sync.dma_start(kt[:, kb[sl]:kb[sl + 1]],
                                          kT[pair][:, kb[sl]:kb[sl + 1]])
                        nc.scalar.dma_start(qt[:, qb[sl]:qb[sl + 1]],
                                            qT[pair][:, qb[sl]:qb[sl + 1]])
                    nc.gpsimd.dma_start(vt[0][:, 5:c], vv[2 * pair][:, 5:c])
                    nc.gpsimd.dma_start(vt[1][:, 5:c], vv[2 * pair + 1][:, 5:c])

            # prefetch step fn: called at group boundaries of the previous
            # pair; one DMA config each so the queues stay busy end-to-end
            def prefetch_step(pair, step):
                # kt -> sync queue, qt -> scalar queue, v -> gpsimd queue:
                # each queue carries ~1.1MB per pair cycle, well under its
                # bandwidth, so loads always finish before the pair starts
                qt, kt, vt = qts[pair], kts[pair], vts[pair]
                if step == 0:
                    nc.sync.dma_start(kt[:, :], kT[pair][:, :])
                elif step == 1:
                    nc.gpsimd.dma_start(vt[0][:, :], vv[2 * pair][:, :])
                elif step == 2:
                    nc.scalar.dma_start(qt[:, :], qT[pair][:, :])
                elif step == 3:
                    nc.gpsimd.dma_start(vt[1][:, :], vv[2 * pair + 1][:, :])

            # prefetch after pair-0's own (finer) slices have queue priority
            PREFETCH_AT = {5: 0, 7: 1, 9: 2, 11: 3}  # gi -> prefetch step

            load_pair(0, startup=True)

            # evac batch b (EB windows) -> store slice index
            B2S_STD = [0, 0, 0, 0, 1, 1, 1, 2, 2, 2, 3]
            B2S_LAST = [0, 0, 0, 0, 1, 1, 1, 2, 3, 4, 5]

            for pair in range(pairs):
                qt, kt, vt = qts[pair], kts[pair], vts[pair]
                stg = stgs[pair]
                accum = {}  # batch -> merged psum accumulation tile

                last_pair = pair == pairs - 1
                b2s = B2S_LAST if last_pair else B2S_STD
                slices = SLICES_LAST if last_pair else SLICES_STD

                def emit_evac(b):
                    # evacuate UNNORMALIZED psum (out cols + denominator l
                    # per head) as fp16; the host does out/l
                    nbw = min(EB, w - b * EB)
                    acc = accum.pop(b)
                    si = b2s[b]
                    s0, s1 = slices[si]
                    nc.vector.tensor_copy(
                        stg[si][:, b * EB - s0:b * EB - s0 + nbw],
                        acc[:, 0:nbw, :])
                    if b == nb - 1 or b2s[b + 1] != si:
                        # store the slice once its last batch is evacuated;
                        # the last pair's small final tiles go out on the
                        # idle sync queue
                        seng = nc.sync if (last_pair and si >= 2) else nc.gpsimd
                        seng.dma_start(out[pair][:, s0:s1], stg[si])

                groups = [list(range(g, min(g + GROUP, c)))
                          for g in range(0, c, GROUP)]
                pending_mm2 = []

                def do_mm2s(chunks, pt):
                    # start=True clears has_written for the WHOLE bank, so
                    # with both heads sharing a bank only the batch's very
                    # first matmul may use it; all later writes rely on the
                    # per-element has_written bits (overwrite where clear,
                    # accumulate where set).
                    for s, p in enumerate(chunks):
                        for h in range(2):
                            col = h * (GROUP * 256) + s * 256
                            if p >= 1:
                                # window p-1 self-contribution (stop)
                                wi = p - 1
                                t = accum[wi // EB]
                                nc.tensor.matmul(
                                    t[:, wi % EB, 65 * h:65 * h + 65],
                                    pt[:, col:col + WS],
                                    vt[h][:, p, :],
                                    start=False, stop=True,
                                )
                                if h == 1 and (wi % EB == EB - 1 or wi == w - 1):
                                    emit_evac(wi // EB)
                            if p <= w - 1:
                                # window p prev-contribution
                                bcol = col + (WS if p >= 1 else 0)
                                t = accum.get(p // EB)
                                first = False
                                if t is None:
                                    t = ps_o.tile([128, EB, 2 * (D + 1)], f32,
                                                  tag="out",
                                                  name=f"acc_{pair}_{p // EB}")
                                    accum[p // EB] = t
                                    first = True
                                nc.tensor.matmul(
                                    t[:, p % EB, 65 * h:65 * h + 65],
                                    pt[:, bcol:bcol + WS],
                                    vt[h][:, p, :],
                                    start=first, stop=False,
                                )

                sch_off = SCH_OFF_BY_PAIR[pair % len(SCH_OFF_BY_PAIR)]
                for gi, chunks in enumerate(groups):
                    if pair + 1 < pairs and gi in PREFETCH_AT:
                        prefetch_step(pair + 1, PREFETCH_AT[gi])
                    ps = ps_s.tile([128, GROUP * 2 * 256], f32, tag="scores")
                    # MM1s
                    for s, p in enumerate(chunks):
                        qlo = max(0, (p - 1) * WS)
                        qhi = min(n, (p + 1) * WS)
                        if p == 0:
                            qhi = min(n, 2 * WS)  # avoid garbage: fill 256
                        nq = qhi - qlo
                        for h in range(2):
                            col = h * (GROUP * 256) + s * 256
                            nc.tensor.matmul(
                                ps[:, col:col + nq],
                                kt[64 * h:64 * h + 64, p * WS:(p + 1) * WS],
                                qt[64 * h:64 * h + 64, qlo:qhi],
                                start=True, stop=True,
                            )
                    # MM2s deferred two groups keep MM1(g+1) ahead of the
                    # exp; they (and their evacs) are emitted BEFORE this
                    # group's exp so an evac lands at the head of the DVE
                    # queue instead of behind a 1.2us Schraudolph - the
                    # PE's next accumulator bank WAR-waits on that evac.
                    depth = 2 if gi < len(groups) - 2 else 1
                    while len(pending_mm2) >= depth:
                        do_mm2s(*pending_mm2.pop(0))
                    # one full-tile exp; garbage cols (last chunk's upper
                    # half) are exp'd but never consumed by MM2.  The last
                    # (single-chunk) group only feeds MM2 from cols 0:128
                    # and 512:640 - skip the trailing garbage.
                    ncols = 640 if len(chunks) == 1 else GROUP * 2 * 256
                    if gi in sch_off:
                        pt = pt_pool.tile([128, GROUP * 2 * 256],
                                          mybir.dt.bfloat16, tag="pt")
                        nc.vector.tensor_scalar(
                            pt.bitcast(mybir.dt.int16)[:, 0:ncols],
                            ps[:, 0:ncols], SCH_A, SCH_B,
                            mybir.AluOpType.mult, mybir.AluOpType.add)
                    else:
                        pt = pt_pool.tile([128, GROUP * 2 * 256], MM2_DT,
                                          tag="pt")
                        nc.scalar.activation(pt[:, 0:ncols], ps[:, 0:ncols],
                                             Exp, scale=SCALE)
                    pending_mm2.append((chunks, pt))
                while pending_mm2:
                    do_mm2s(*pending_mm2.pop(0))

    nc.compile()
    return nc


def _get_nc():
    if "nc" not in _NC_CACHE:
        _NC_CACHE["nc"] = build_nc()
    return _NC_CACHE["nc"]


def _prep_core(qf, kf, vf, lo):
    """Build one core's input dict from flat [64, 4096, 64] fp32 arrays."""
    q8 = qf[lo:lo + HPC]                      # [8, 4096, 64]
    k8 = kf[lo:lo + HPC]
    v8 = vf[lo:lo + HPC]

    qT = np.ascontiguousarray(q8.transpose(0, 2, 1)).reshape(PAIRS, 128, N)
    qT = qT.astype(np.float16)

    pad = np.full((HPC, WS, D), -1.0, dtype=np.float32)
    kp = np.concatenate([pad, k8], axis=1)    # [8, 4224, 64]
    kT = np.ascontiguousarray(kp.transpose(0, 2, 1)).reshape(PAIRS, 128, C * WS)
    kT = kT.astype(np.float16)

    vp = np.concatenate([pad, v8], axis=1)    # [8, 4224, 64]
    ones = np.ones((HPC, C * WS, 1), dtype=np.float32)
    va = np.concatenate([vp, ones], axis=2)   # [8, 4224, 65]
    va = va.reshape(HPC, C, WS, D + 1).transpose(0, 2, 1, 3)  # [8, 128, 33, 65]
    va = np.ascontiguousarray(va).astype(np.float16)

    return {"qT": qT, "kT": kT, "v": va}


def kernel(q, k, v):
    q = np.asarray(q, dtype=np.float32)
    k = np.asarray(k, dtype=np.float32)
    v = np.asarray(v, dtype=np.float32)
    qf = q.reshape(B * H, N, D)
    kf = k.reshape(B * H, N, D)
    vf = v.reshape(B * H, N, D)

    nc = _get_nc()
    in_maps = [_prep_core(qf, kf, vf, HPC * c) for c in range(NC)]
    res = run_bass_kernel_spmd(nc, in_maps, core_ids=list(range(NC)))

    outs = []
    for c in range(NC):
        o = res.results[c]["out"].astype(np.float32)  # [4, 128, 32, 130]
        o = o.reshape(PAIRS, 128, W, 2, D + 1)
        o = o[..., :D] / o[..., D:]                   # host-side normalize
        # [pair, q, w, h, d] -> [pair, h, w, q, d] -> [8 heads, 4096, 64]
        o = o.transpose(0, 3, 2, 1, 4).reshape(HPC, N, D)
        outs.append(o)
    return np.concatenate(outs, axis=0).reshape(B, H, N, D).astype(np.float32)


if __name__ == "__main__":
    rng = np.random.default_rng(0)
    q = rng.standard_normal((B, H, N, D), dtype=np.float32)
    k = rng.standard_normal((B, H, N, D), dtype=np.float32)
    v = rng.standard_normal((B, H, N, D), dtype=np.float32)
    o = kernel(q, k, v)
    print("out", o.shape, o.dtype, float(np.abs(o).max()))


# revision 27
# speedup vs baseline: 1.0577x; 1.0577x over previous
"""Local (windowed) attention Trainium2 Bass kernel.

Problem: q,k,v [8, 8, 4096, 64] fp32; window 128, look_backward 1, pad -1.0.
out[b,h,w,i,:] = softmax(scale * q_wi . [k_{w-1}; k_w]) @ [v_{w-1}; v_w]
(with window -1 = all -1.0 pad values, which DO enter the softmax).

Sharding: data-parallel over flat batch*heads (64) -> 8 heads per core.

Per-core layouts (prepared host-side, all fp16 -- fp8 variants measured
over the 2e-2 error gate host-side, so inputs stay 16-bit):
  qT : [4, 128, 4096]  head pair stacked on partitions (d=64 each),
                       free axis = 4096 queries (d-major transposed)
  kT : [4, 128, 4224]  same, one pad chunk (128 keys of -1.0) prepended
                       -> 33 chunks of 128 keys
  v  : [8, 128, 33, 65] per head; partition = key-within-chunk, pad chunk
                       prepended; col 64 = 1.0 (ones col yields softmax l)
  out: [4, 128, 32, 130] UNNORMALIZED out cols for both heads of a pair
                       interleaved per window ([w, h, 65]; col 64 of each
                       head's 65 = denominator l); host divides + unpacks.

Device pipeline per head pair, per key-chunk group (2 chunks):
  MM1 (fp16): scoresT[j, i] per chunk/head; heads alternate PE row-group
              bases 0/64 (run concurrently); psum layout [h0:512|h1:512]
              so each bank sees a single weight base partition (mixing
              bases within a bank hard-crashes the device).
  exp: one full-tile op per group, psum fp32 -> 16-bit P tiles.  Split
       across Act (exact table exp) and DVE (Schraudolph exp2 writing
       bf16 bits via an int16 bitcast) to share the 1 elem/cycle/lane
       elementwise throughput wall; the split ratio is the engine
       balance knob.
  MM2 (16b): out_w[i, 0:65] += P_blockT @ v_aug[p] into merged-head psum
       accumulators [128, 3, 130] (one 2KB bank holds 3 windows x both
       heads), deferred two groups behind MM1 so exp never waits on PE.
  evac: one psum->fp16 copy per 3-window batch covering both heads
       (Act and DVE alternate by a balance set); stores stream out on
       the gpsimd queue in 3 slices per pair.

DMA: kt+qt ride the sync HWDGE queue, v + output stores the gpsimd
SWDGE queue; pair p+1's loads are prefetched during pair p's compute so
the PE/exp engines never see a pair-boundary stall (also keeps the PE's
HAM clock-gate warm).

Accuracy: ~1.1e-2 relative vs the 2e-2 gate (host-simulated and HW
measured): Schraudolph mantissa interpolation (~3%/elem, mostly
cancelled by softmax renormalization) on the DVE share; fp16 operands
contribute ~6e-4.
"""

import os
import sys

for _p in ("/opt/trn_rl_repo", "/opt/pypackages"):
    if os.path.isdir(_p) and _p not in sys.path:
        sys.path.append(_p)

import numpy as np

import concourse.mybir as mybir
import concourse.tile as tile
from concourse import bacc
from concourse.bass_utils import run_bass_kernel_spmd

B, H, N, D = 8, 8, 4096, 64
WS = 128                 # window size
W = N // WS              # 32 windows
C = W + 1                # 33 key chunks incl. pad chunk
NC = 8                   # cores
HPC = (B * H) // NC      # 8 heads per core
PAIRS = HPC // 2         # 4 head pairs per core
SCALE = float(D) ** -0.5

MM1_DT = mybir.dt.float16
MM2_DT = mybir.dt.float16
GROUP = 2                # key chunks per exp batch
EB = 3                   # windows per merged-head psum accumulator bank
                         # (3 * 130 * 4B = 1560 <= 2KB)
NB = (W + EB - 1) // EB  # 11 evac batches per pair (last has 2 windows)

# Schraudolph exp2 offload: for a subset of groups the exp runs on the DVE
# instead of the Act engine, directly in the bf16 bit domain:
#   int16 t = round(s * (128*log2e*SCALE) + (16256 - C))
# t's bits ARE bf16(2^(s*log2e*SCALE)) with linear mantissa interpolation
# (~3% deterministic error); softmax renormalization cancels most of it.
# Exp-engine assignment per pair (group indices exp'd on the DVE, rest
# on Act).  6 of 17 groups (incl. the cheap trimmed last one) on DVE
# plus all evacs balances the two engines at ~12.5us per pair (the
# elementwise throughput wall).
SCH_OFF_BY_PAIR = [
    {2, 5, 8, 11, 14, 16},
    {2, 5, 8, 11, 14, 16},
    {2, 5, 8, 11, 14, 16},
    {2, 5, 8, 11, 14, 16},
]
SCH_A = float(128.0 / np.log(2.0) * SCALE)
SCH_B = float(16256.0 - 5.59)

_NC_CACHE = {}


def build_nc(pairs=PAIRS, w=W):
    c = w + 1
    n = w * WS
    nb = (w + EB - 1) // EB
    nc = bacc.Bacc("TRN2", target_bir_lowering=False)
    qT = nc.dram_tensor("qT", [pairs, 128, n], MM1_DT, kind="ExternalInput")
    kT = nc.dram_tensor("kT", [pairs, 128, c * WS], MM1_DT, kind="ExternalInput")
    vv = nc.dram_tensor("v", [2 * pairs, 128, c, D + 1], MM2_DT, kind="ExternalInput")
    out = nc.dram_tensor("out", [pairs, 128, w, 2 * (D + 1)], mybir.dt.float16,
                         kind="ExternalOutput")

    f32 = mybir.dt.float32
    Exp = mybir.ActivationFunctionType.Exp

    with tile.TileContext(nc) as tc:
        with (
            tc.tile_pool(name="qk", bufs=2) as qk_pool,
            tc.tile_pool(name="vp", bufs=4) as v_pool,
            tc.tile_pool(name="pt", bufs=4) as pt_pool,
            tc.tile_pool(name="st", bufs=2) as st_pool,
            tc.tile_pool(name="ps_s", bufs=3, space="PSUM") as ps_s,
            tc.tile_pool(name="ps_o", bufs=2, space="PSUM") as ps_o,
        ):
            # --- tiles for all pairs, loads for pair 0 up front ---------
            # stg is split into one tile per output store slice so an
            # evac never WAR-waits on a still-in-flight DMA store of an
            # earlier slice (DMA reads hold the whole tile).  The last
            # pair stores batch-by-batch (whole small tiles) so the
            # post-compute drain tail is minimal.
            SLICES_STD = [(0, 12), (12, 21), (21, 30), (30, w)]
            SLICES_LAST = [(0, 12), (12, 21), (21, 24), (24, 27), (27, 30),
                           (30, w)]
            qts, kts, vts, stgs = [], [], [], []
            for pair in range(pairs):
                qts.append(qk_pool.tile([128, n], MM1_DT, tag="qT",
                                        name=f"qt_{pair}"))
                kts.append(qk_pool.tile([128, c * WS], MM1_DT, tag="kT",
                                        name=f"kt_{pair}"))
                vts.append([v_pool.tile([128, c, D + 1], MM2_DT, tag="v",
                                        name=f"v_{pair}_{h}") for h in range(2)])
                slices = SLICES_LAST if pair == pairs - 1 else SLICES_STD
                stgs.append([st_pool.tile([128, e - s, 2 * (D + 1)],
                                          mybir.dt.float16, tag=f"stg{si}",
                                          name=f"stg_{pair}_{si}")
                             for si, (s, e) in enumerate(slices)])

            def load_pair(pair, startup=False):
                qt, kt, vt = qts[pair], kts[pair], vts[pair]
                if startup:
                    # fine-grained need-ordered first slices so MM1 starts
                    # ~1.5us in; qt rides the scalar HWDGE queue (Act is
                    # idle during startup) so kt/qt stream in parallel
                    kb = [0, 256, 1024, 2304, c * WS]
                    qb = [0, 384, 1024, 2304, n]
                    nc.sync.dma_start(kt[:, kb[0]:kb[1]], kT[pair][:, kb[0]:kb[1]])
                    nc.scalar.dma_start(qt[:, qb[0]:qb[1]], qT[pair][:, qb[0]:qb[1]])
                    nc.gpsimd.dma_start(vt[0][:, 0:5], vv[2 * pair][:, 0:5])
                    nc.gpsimd.dma_start(vt[1][:, 0:5], vv[2 * pair + 1][:, 0:5])
                    for sl in (1, 2, 3):
                        nc.sync.dma_start(kt[:, kb[sl]:kb[sl + 1]],
                                          kT[pair][:, kb[sl]:kb[sl + 1]])
                        nc.scalar.dma_start(qt[:, qb[sl]:qb[sl + 1]],
                                            qT[pair][:, qb[sl]:qb[sl + 1]])
                    # keep both heads' v in lockstep with chunk consumption
                    # (a late head-1 slice head-of-line-blocks the PE FIFO
                    # at that head's MM2s)
                    for lo, hi in ((5, 19), (19, c)):
                        nc.gpsimd.dma_start(vt[0][:, lo:hi],
                                            vv[2 * pair][:, lo:hi])
                        nc.gpsimd.dma_start(vt[1][:, lo:hi],
                                            vv[2 * pair + 1][:, lo:hi])

            # prefetch step fn: called at group boundaries of the previous
            # pair; one DMA config each so the queues stay busy end-to-end
            def prefetch_step(pair, step):
                # kt -> sync queue, qt -> scalar queue, v -> gpsimd queue:
                # each queue carries ~1.1MB per pair cycle, well under its
                # bandwidth, so loads always finish before the pair starts
                qt, kt, vt = qts[pair], kts[pair], vts[pair]
                if step == 0:
                    nc.sync.dma_start(kt[:, :], kT[pair][:, :])
                elif step == 1:
                    nc.gpsimd.dma_start(vt[0][:, :], vv[2 * pair][:, :])
                elif step == 2:
                    nc.scalar.dma_start(qt[:, :], qT[pair][:, :])
                elif step == 3:
                    nc.gpsimd.dma_start(vt[1][:, :], vv[2 * pair + 1][:, :])

            # prefetch after pair-0's own (finer) slices have queue priority
            PREFETCH_AT = {5: 0, 7: 1, 9: 2, 11: 3}  # gi -> prefetch step

            load_pair(0, startup=True)

            # evac batch b (EB windows) -> store slice index
            B2S_STD = [0, 0, 0, 0, 1, 1, 1, 2, 2, 2, 3]
            B2S_LAST = [0, 0, 0, 0, 1, 1, 1, 2, 3, 4, 5]

            for pair in range(pairs):
                qt, kt, vt = qts[pair], kts[pair], vts[pair]
                stg = stgs[pair]
                accum = {}  # batch -> merged psum accumulation tile

                last_pair = pair == pairs - 1
                b2s = B2S_LAST if last_pair else B2S_STD
                slices = SLICES_LAST if last_pair else SLICES_STD

                def emit_evac(b):
                    # evacuate UNNORMALIZED psum (out cols + denominator l
                    # per head) as fp16; the host does out/l
                    nbw = min(EB, w - b * EB)
                    acc = accum.pop(b)
                    si = b2s[b]
                    s0, s1 = slices[si]
                    nc.vector.tensor_copy(
                        stg[si][:, b * EB - s0:b * EB - s0 + nbw],
                        acc[:, 0:nbw, :])
                    if b == nb - 1 or b2s[b + 1] != si:
                        # store the slice once its last batch is evacuated;
                        # the last pair's small final tiles go out on the
                        # idle sync queue
                        seng = nc.sync if (last_pair and si >= 2) else nc.gpsimd
                        seng.dma_start(out[pair][:, s0:s1], stg[si])

                groups = [list(range(g, min(g + GROUP, c)))
                          for g in range(0, c, GROUP)]
                pending_mm2 = []

                def do_mm2s(chunks, pt):
                    # start=True clears has_written for the WHOLE bank, so
                    # with both heads sharing a bank only the batch's very
                    # first matmul may use it; all later writes rely on the
                    # per-element has_written bits (overwrite where clear,
                    # accumulate where set).
                    for s, p in enumerate(chunks):
                        for h in range(2):
                            col = h * (GROUP * 256) + s * 256
                            if p >= 1:
                                # window p-1 self-contribution (stop)
                                wi = p - 1
                                t = accum[wi // EB]
                                nc.tensor.matmul(
                                    t[:, wi % EB, 65 * h:65 * h + 65],
                                    pt[:, col:col + WS],
                                    vt[h][:, p, :],
                                    start=False, stop=True,
                                )
                                if h == 1 and (wi % EB == EB - 1 or wi == w - 1):
                                    emit_evac(wi // EB)
                            if p <= w - 1:
                                # window p prev-contribution
                                bcol = col + (WS if p >= 1 else 0)
                                t = accum.get(p // EB)
                                first = False
                                if t is None:
                                    t = ps_o.tile([128, EB, 2 * (D + 1)], f32,
                                                  tag="out",
                                                  name=f"acc_{pair}_{p // EB}")
                                    accum[p // EB] = t
                                    first = True
                                nc.tensor.matmul(
                                    t[:, p % EB, 65 * h:65 * h + 65],
                                    pt[:, bcol:bcol + WS],
                                    vt[h][:, p, :],
                                    start=first, stop=False,
                                )

                sch_off = SCH_OFF_BY_PAIR[pair % len(SCH_OFF_BY_PAIR)]
                for gi, chunks in enumerate(groups):
                    if pair + 1 < pairs and gi in PREFETCH_AT:
                        prefetch_step(pair + 1, PREFETCH_AT[gi])
                    ps = ps_s.tile([128, GROUP * 2 * 256], f32, tag="scores")
                    # MM1s
                    for s, p in enumerate(chunks):
                        qlo = max(0, (p - 1) * WS)
                        qhi = min(n, (p + 1) * WS)
                        if p == 0:
                            qhi = min(n, 2 * WS)  # avoid garbage: fill 256
                        nq = qhi - qlo
                        for h in range(2):
                            col = h * (GROUP * 256) + s * 256
                            nc.tensor.matmul(
                                ps[:, col:col + nq],
                                kt[64 * h:64 * h + 64, p * WS:(p + 1) * WS],
                                qt[64 * h:64 * h + 64, qlo:qhi],
                                start=True, stop=True,
                            )
                    # one full-tile exp; garbage cols (last chunk's upper
                    # half) are exp'd but never consumed by MM2.  The last
                    # (single-chunk) group only feeds MM2 from cols 0:128
                    # and 512:640 - skip the trailing garbage.
                    ncols = 640 if len(chunks) == 1 else GROUP * 2 * 256
                    if gi in sch_off:
                        pt = pt_pool.tile([128, GROUP * 2 * 256],
                                          mybir.dt.bfloat16, tag="pt")
                        nc.vector.tensor_scalar(
                            pt.bitcast(mybir.dt.int16)[:, 0:ncols],
                            ps[:, 0:ncols], SCH_A, SCH_B,
                            mybir.AluOpType.mult, mybir.AluOpType.add)
                    else:
                        pt = pt_pool.tile([128, GROUP * 2 * 256], MM2_DT,
                                          tag="pt")
                        nc.scalar.activation(pt[:, 0:ncols], ps[:, 0:ncols],
                                             Exp, scale=SCALE)
                    # MM2s deferred two groups: keeps MM1(g+1) ahead of the
                    # Act/DVE exp so the exp engines never wait on the PE.
                    # Shallower near the end so the drain tail is short.
                    pending_mm2.append((chunks, pt))
                    depth = 2 if gi < len(groups) - 2 else 1
                    if len(pending_mm2) > depth:
                        do_mm2s(*pending_mm2.pop(0))
                while pending_mm2:
                    do_mm2s(*pending_mm2.pop(0))

    nc.compile()
    return nc


def _get_nc():
    if "nc" not in _NC_CACHE:
        _NC_CACHE["nc"] = build_nc()
    return _NC_CACHE["nc"]


def _prep_core(qf, kf, vf, lo):
    """Build one core's input dict from flat [64, 4096, 64] fp32 arrays."""
    q8 = qf[lo:lo + HPC]                      # [8, 4096, 64]
    k8 = kf[lo:lo + HPC]
    v8 = vf[lo:lo + HPC]

    qT = np.ascontiguousarray(q8.transpose(0, 2, 1)).reshape(PAIRS, 128, N)
    qT = qT.astype(np.float16)

    pad = np.full((HPC, WS, D), -1.0, dtype=np.float32)
    kp = np.concatenate([pad, k8], axis=1)    # [8, 4224, 64]
    kT = np.ascontiguousarray(kp.transpose(0, 2, 1)).reshape(PAIRS, 128, C * WS)
    kT = kT.astype(np.float16)

    vp = np.concatenate([pad, v8], axis=1)    # [8, 4224, 64]
    ones = np.ones((HPC, C * WS, 1), dtype=np.float32)
    va = np.concatenate([vp, ones], axis=2)   # [8, 4224, 65]
    va = va.reshape(HPC, C, WS, D + 1).transpose(0, 2, 1, 3)  # [8, 128, 33, 65]
    va = np.ascontiguousarray(va).astype(np.float16)

    return {"qT": qT, "kT": kT, "v": va}


def kernel(q, k, v):
    q = np.asarray(q, dtype=np.float32)
    k = np.asarray(k, dtype=np.float32)
    v = np.asarray(v, dtype=np.float32)
    qf = q.reshape(B * H, N, D)
    kf = k.reshape(B * H, N, D)
    vf = v.reshape(B * H, N, D)

    nc = _get_nc()
    in_maps = [_prep_core(qf, kf, vf, HPC * c) for c in range(NC)]
    res = run_bass_kernel_spmd(nc, in_maps, core_ids=list(range(NC)))

    outs = []
    for c in range(NC):
        o = res.results[c]["out"].astype(np.float32)  # [4, 128, 32, 130]
        o = o.reshape(PAIRS, 128, W, 2, D + 1)
        o = o[..., :D] / o[..., D:]                   # host-side normalize
        # [pair, q, w, h, d] -> [pair, h, w, q, d] -> [8 heads, 4096, 64]
        o = o.transpose(0, 3, 2, 1, 4).reshape(HPC, N, D)
        outs.append(o)
    return np.concatenate(outs, axis=0).reshape(B, H, N, D).astype(np.float32)


if __name__ == "__main__":
    rng = np.random.default_rng(0)
    q = rng.standard_normal((B, H, N, D), dtype=np.float32)
    k = rng.standard_normal((B, H, N, D), dtype=np.float32)
    v = rng.standard_normal((B, H, N, D), dtype=np.float32)
    o = kernel(q, k, v)
    print("out", o.shape, o.dtype, float(np.abs(o).max()))


# revision 28
# speedup vs baseline: 1.0720x; 1.0135x over previous
"""Local (windowed) attention Trainium2 Bass kernel.

Problem: q,k,v [8, 8, 4096, 64] fp32; window 128, look_backward 1, pad -1.0.
out[b,h,w,i,:] = softmax(scale * q_wi . [k_{w-1}; k_w]) @ [v_{w-1}; v_w]
(with window -1 = all -1.0 pad values, which DO enter the softmax).

Sharding: data-parallel over flat batch*heads (64) -> 8 heads per core.

Per-core layouts (prepared host-side, all fp16 -- fp8 variants measured
over the 2e-2 error gate host-side, so inputs stay 16-bit):
  qT : [4, 128, 4096]  head pair stacked on partitions (d=64 each),
                       free axis = 4096 queries (d-major transposed)
  kT : [4, 128, 4224]  same, one pad chunk (128 keys of -1.0) prepended
                       -> 33 chunks of 128 keys
  v  : [8, 128, 33, 65] per head; partition = key-within-chunk, pad chunk
                       prepended; col 64 = 1.0 (ones col yields softmax l)
  out: [4, 128, 32, 130] UNNORMALIZED out cols for both heads of a pair
                       interleaved per window ([w, h, 65]; col 64 of each
                       head's 65 = denominator l); host divides + unpacks.

Device pipeline per head pair, per key-chunk group (2 chunks):
  MM1 (fp16): scoresT[j, i] per chunk/head; heads alternate PE row-group
              bases 0/64 (run concurrently); psum layout [h0:512|h1:512]
              so each bank sees a single weight base partition (mixing
              bases within a bank hard-crashes the device).
  exp: one full-tile op per group, psum fp32 -> 16-bit P tiles.  Split
       across Act (exact table exp) and DVE (Schraudolph exp2 writing
       bf16 bits via an int16 bitcast) to share the 1 elem/cycle/lane
       elementwise throughput wall; the split ratio is the engine
       balance knob.
  MM2 (16b): out_w[i, 0:65] += P_blockT @ v_aug[p] into merged-head psum
       accumulators [128, 3, 130] (one 2KB bank holds 3 windows x both
       heads), deferred two groups behind MM1 so exp never waits on PE.
  evac: one psum->fp16 copy per 3-window batch covering both heads
       (Act and DVE alternate by a balance set); stores stream out on
       the gpsimd queue in 3 slices per pair.

DMA: kt+qt ride the sync HWDGE queue, v + output stores the gpsimd
SWDGE queue; pair p+1's loads are prefetched during pair p's compute so
the PE/exp engines never see a pair-boundary stall (also keeps the PE's
HAM clock-gate warm).

Accuracy: ~1.1e-2 relative vs the 2e-2 gate (host-simulated and HW
measured): Schraudolph mantissa interpolation (~3%/elem, mostly
cancelled by softmax renormalization) on the DVE share; fp16 operands
contribute ~6e-4.
"""

import os
import sys

for _p in ("/opt/trn_rl_repo", "/opt/pypackages"):
    if os.path.isdir(_p) and _p not in sys.path:
        sys.path.append(_p)

import numpy as np

import concourse.mybir as mybir
import concourse.tile as tile
from concourse import bacc
from concourse.bass_utils import run_bass_kernel_spmd

B, H, N, D = 8, 8, 4096, 64
WS = 128                 # window size
W = N // WS              # 32 windows
C = W + 1                # 33 key chunks incl. pad chunk
NC = 8                   # cores
HPC = (B * H) // NC      # 8 heads per core
PAIRS = HPC // 2         # 4 head pairs per core
SCALE = float(D) ** -0.5

MM1_DT = mybir.dt.float16
MM2_DT = mybir.dt.float16
GROUP = 2                # key chunks per exp batch
EB = 3                   # windows per merged-head psum accumulator bank
                         # (3 * 130 * 4B = 1560 <= 2KB)
NB = (W + EB - 1) // EB  # 11 evac batches per pair (last has 2 windows)

# Schraudolph exp2 offload: for a subset of groups the exp runs on the DVE
# instead of the Act engine, directly in the bf16 bit domain:
#   int16 t = round(s * (128*log2e*SCALE) + (16256 - C))
# t's bits ARE bf16(2^(s*log2e*SCALE)) with linear mantissa interpolation
# (~3% deterministic error); softmax renormalization cancels most of it.
# Exp-engine assignment per pair (group indices exp'd on the DVE, rest
# on Act).  6 of 17 groups (incl. the cheap trimmed last one) on DVE
# plus all evacs balances the two engines at ~12.5us per pair (the
# elementwise throughput wall).
SCH_OFF_BY_PAIR = [
    {2, 5, 8, 11, 14, 16},
    {2, 5, 8, 11, 14, 16},
    {2, 5, 8, 11, 14, 16},
    {2, 5, 8, 11, 14, 16},
]
SCH_A = float(128.0 / np.log(2.0) * SCALE)
SCH_B = float(16256.0 - 5.59)

_NC_CACHE = {}


def build_nc(pairs=PAIRS, w=W):
    c = w + 1
    n = w * WS
    nb = (w + EB - 1) // EB
    nc = bacc.Bacc("TRN2", target_bir_lowering=False)
    qT = nc.dram_tensor("qT", [pairs, 128, n], MM1_DT, kind="ExternalInput")
    kT = nc.dram_tensor("kT", [pairs, 128, c * WS], MM1_DT, kind="ExternalInput")
    vv = nc.dram_tensor("v", [2 * pairs, 128, c, D + 1], MM2_DT, kind="ExternalInput")
    out = nc.dram_tensor("out", [pairs, 128, w, 2 * (D + 1)], mybir.dt.float16,
                         kind="ExternalOutput")

    f32 = mybir.dt.float32
    Exp = mybir.ActivationFunctionType.Exp

    with tile.TileContext(nc) as tc:
        with (
            tc.tile_pool(name="qk", bufs=2) as qk_pool,
            tc.tile_pool(name="vp", bufs=4) as v_pool,
            tc.tile_pool(name="pt", bufs=4) as pt_pool,
            tc.tile_pool(name="st", bufs=2) as st_pool,
            tc.tile_pool(name="ps_s", bufs=3, space="PSUM") as ps_s,
            tc.tile_pool(name="ps_o", bufs=2, space="PSUM") as ps_o,
        ):
            # --- tiles for all pairs, loads for pair 0 up front ---------
            # stg is split into one tile per output store slice so an
            # evac never WAR-waits on a still-in-flight DMA store of an
            # earlier slice (DMA reads hold the whole tile).  The last
            # pair stores batch-by-batch (whole small tiles) so the
            # post-compute drain tail is minimal.
            SLICES_STD = [(0, 12), (12, 21), (21, 30), (30, w)]
            SLICES_LAST = [(0, 12), (12, 21), (21, 24), (24, 27), (27, 30),
                           (30, w)]
            qts, kts, vts, stgs = [], [], [], []
            for pair in range(pairs):
                qts.append(qk_pool.tile([128, n], MM1_DT, tag="qT",
                                        name=f"qt_{pair}"))
                kts.append(qk_pool.tile([128, c * WS], MM1_DT, tag="kT",
                                        name=f"kt_{pair}"))
                vts.append([v_pool.tile([128, c, D + 1], MM2_DT, tag="v",
                                        name=f"v_{pair}_{h}") for h in range(2)])
                slices = SLICES_LAST if pair == pairs - 1 else SLICES_STD
                stgs.append([st_pool.tile([128, e - s, 2 * (D + 1)],
                                          mybir.dt.float16, tag=f"stg{si}",
                                          name=f"stg_{pair}_{si}")
                             for si, (s, e) in enumerate(slices)])

            def load_pair(pair, startup=False):
                qt, kt, vt = qts[pair], kts[pair], vts[pair]
                if startup:
                    # fine-grained need-ordered first slices so MM1 starts
                    # ~1.5us in; qt rides the scalar HWDGE queue (Act is
                    # idle during startup) so kt/qt stream in parallel
                    kb = [0, 256, 1024, 2304, c * WS]
                    qb = [0, 384, 1024, 2304, n]
                    nc.sync.dma_start(kt[:, kb[0]:kb[1]], kT[pair][:, kb[0]:kb[1]])
                    nc.scalar.dma_start(qt[:, qb[0]:qb[1]], qT[pair][:, qb[0]:qb[1]])
                    nc.gpsimd.dma_start(vt[0][:, 0:5], vv[2 * pair][:, 0:5])
                    nc.gpsimd.dma_start(vt[1][:, 0:5], vv[2 * pair + 1][:, 0:5])
                    for sl in (1, 2, 3):
                        nc.sync.dma_start(kt[:, kb[sl]:kb[sl + 1]],
                                          kT[pair][:, kb[sl]:kb[sl + 1]])
                        nc.scalar.dma_start(qt[:, qb[sl]:qb[sl + 1]],
                                            qT[pair][:, qb[sl]:qb[sl + 1]])
                    # keep both heads' v in lockstep with chunk consumption
                    # (a late head-1 slice head-of-line-blocks the PE FIFO
                    # at that head's MM2s)
                    for lo, hi in ((5, 19), (19, c)):
                        nc.gpsimd.dma_start(vt[0][:, lo:hi],
                                            vv[2 * pair][:, lo:hi])
                        nc.gpsimd.dma_start(vt[1][:, lo:hi],
                                            vv[2 * pair + 1][:, lo:hi])

            # prefetch step fn: called at group boundaries of the previous
            # pair; one DMA config each so the queues stay busy end-to-end
            def prefetch_step(pair, step):
                # kt -> sync queue, qt -> scalar queue, v -> gpsimd queue:
                # each queue carries ~1.1MB per pair cycle, well under its
                # bandwidth, so loads always finish before the pair starts
                qt, kt, vt = qts[pair], kts[pair], vts[pair]
                if step == 0:
                    nc.sync.dma_start(kt[:, :], kT[pair][:, :])
                elif step == 1:
                    nc.gpsimd.dma_start(vt[0][:, :], vv[2 * pair][:, :])
                elif step == 2:
                    nc.scalar.dma_start(qt[:, :], qT[pair][:, :])
                elif step == 3:
                    nc.gpsimd.dma_start(vt[1][:, :], vv[2 * pair + 1][:, :])

            # prefetch after pair-0's own (finer) slices have queue priority;
            # HBM bandwidth is the binding resource during the rampup crunch,
            # so the next pair's loads start only once the current pair's
            # tail slices are nearly done
            PREFETCH_AT = {7: 0, 9: 1, 11: 2, 13: 3}  # gi -> prefetch step

            load_pair(0, startup=True)

            # evac batch b (EB windows) -> store slice index
            B2S_STD = [0, 0, 0, 0, 1, 1, 1, 2, 2, 2, 3]
            B2S_LAST = [0, 0, 0, 0, 1, 1, 1, 2, 3, 4, 5]

            for pair in range(pairs):
                qt, kt, vt = qts[pair], kts[pair], vts[pair]
                stg = stgs[pair]
                accum = {}  # batch -> merged psum accumulation tile

                last_pair = pair == pairs - 1
                b2s = B2S_LAST if last_pair else B2S_STD
                slices = SLICES_LAST if last_pair else SLICES_STD

                def emit_evac(b):
                    # evacuate UNNORMALIZED psum (out cols + denominator l
                    # per head) as fp16; the host does out/l
                    nbw = min(EB, w - b * EB)
                    acc = accum.pop(b)
                    si = b2s[b]
                    s0, s1 = slices[si]
                    nc.vector.tensor_copy(
                        stg[si][:, b * EB - s0:b * EB - s0 + nbw],
                        acc[:, 0:nbw, :])
                    if b == nb - 1 or b2s[b + 1] != si:
                        # store the slice once its last batch is evacuated;
                        # the last pair's small final tiles go out on the
                        # idle sync queue
                        seng = nc.sync if (last_pair and si >= 2) else nc.gpsimd
                        seng.dma_start(out[pair][:, s0:s1], stg[si])

                groups = [list(range(g, min(g + GROUP, c)))
                          for g in range(0, c, GROUP)]
                pending_mm2 = []

                def do_mm2s(chunks, pt):
                    # start=True clears has_written for the WHOLE bank, so
                    # with both heads sharing a bank only the batch's very
                    # first matmul may use it; all later writes rely on the
                    # per-element has_written bits (overwrite where clear,
                    # accumulate where set).
                    for s, p in enumerate(chunks):
                        for h in range(2):
                            col = h * (GROUP * 256) + s * 256
                            if p >= 1:
                                # window p-1 self-contribution (stop)
                                wi = p - 1
                                t = accum[wi // EB]
                                nc.tensor.matmul(
                                    t[:, wi % EB, 65 * h:65 * h + 65],
                                    pt[:, col:col + WS],
                                    vt[h][:, p, :],
                                    start=False, stop=True,
                                )
                                if h == 1 and (wi % EB == EB - 1 or wi == w - 1):
                                    emit_evac(wi // EB)
                            if p <= w - 1:
                                # window p prev-contribution
                                bcol = col + (WS if p >= 1 else 0)
                                t = accum.get(p // EB)
                                first = False
                                if t is None:
                                    t = ps_o.tile([128, EB, 2 * (D + 1)], f32,
                                                  tag="out",
                                                  name=f"acc_{pair}_{p // EB}")
                                    accum[p // EB] = t
                                    first = True
                                nc.tensor.matmul(
                                    t[:, p % EB, 65 * h:65 * h + 65],
                                    pt[:, bcol:bcol + WS],
                                    vt[h][:, p, :],
                                    start=first, stop=False,
                                )

                sch_off = SCH_OFF_BY_PAIR[pair % len(SCH_OFF_BY_PAIR)]
                for gi, chunks in enumerate(groups):
                    if pair + 1 < pairs and gi in PREFETCH_AT:
                        prefetch_step(pair + 1, PREFETCH_AT[gi])
                    ps = ps_s.tile([128, GROUP * 2 * 256], f32, tag="scores")
                    # MM1s
                    for s, p in enumerate(chunks):
                        qlo = max(0, (p - 1) * WS)
                        qhi = min(n, (p + 1) * WS)
                        if p == 0:
                            qhi = min(n, 2 * WS)  # avoid garbage: fill 256
                        nq = qhi - qlo
                        for h in range(2):
                            col = h * (GROUP * 256) + s * 256
                            nc.tensor.matmul(
                                ps[:, col:col + nq],
                                kt[64 * h:64 * h + 64, p * WS:(p + 1) * WS],
                                qt[64 * h:64 * h + 64, qlo:qhi],
                                start=True, stop=True,
                            )
                    # one full-tile exp; garbage cols (last chunk's upper
                    # half) are exp'd but never consumed by MM2.  The last
                    # (single-chunk) group only feeds MM2 from cols 0:128
                    # and 512:640 - skip the trailing garbage.
                    ncols = 640 if len(chunks) == 1 else GROUP * 2 * 256
                    if gi in sch_off:
                        pt = pt_pool.tile([128, GROUP * 2 * 256],
                                          mybir.dt.bfloat16, tag="pt")
                        nc.vector.tensor_scalar(
                            pt.bitcast(mybir.dt.int16)[:, 0:ncols],
                            ps[:, 0:ncols], SCH_A, SCH_B,
                            mybir.AluOpType.mult, mybir.AluOpType.add)
                    else:
                        pt = pt_pool.tile([128, GROUP * 2 * 256], MM2_DT,
                                          tag="pt")
                        nc.scalar.activation(pt[:, 0:ncols], ps[:, 0:ncols],
                                             Exp, scale=SCALE)
                    # MM2s deferred two groups: keeps MM1(g+1) ahead of the
                    # Act/DVE exp so the exp engines never wait on the PE.
                    # Shallower near the end so the drain tail is short.
                    pending_mm2.append((chunks, pt))
                    depth = 2 if gi < len(groups) - 2 else 1
                    if len(pending_mm2) > depth:
                        do_mm2s(*pending_mm2.pop(0))
                while pending_mm2:
                    do_mm2s(*pending_mm2.pop(0))

    nc.compile()
    return nc


def _get_nc():
    if "nc" not in _NC_CACHE:
        _NC_CACHE["nc"] = build_nc()
    return _NC_CACHE["nc"]


def _prep_core(qf, kf, vf, lo):
    """Build one core's input dict from flat [64, 4096, 64] fp32 arrays."""
    q8 = qf[lo:lo + HPC]                      # [8, 4096, 64]
    k8 = kf[lo:lo + HPC]
    v8 = vf[lo:lo + HPC]

    qT = np.ascontiguousarray(q8.transpose(0, 2, 1)).reshape(PAIRS, 128, N)
    qT = qT.astype(np.float16)

    pad = np.full((HPC, WS, D), -1.0, dtype=np.float32)
    kp = np.concatenate([pad, k8], axis=1)    # [8, 4224, 64]
    kT = np.ascontiguousarray(kp.transpose(0, 2, 1)).reshape(PAIRS, 128, C * WS)
    kT = kT.astype(np.float16)

    vp = np.concatenate([pad, v8], axis=1)    # [8, 4224, 64]
    ones = np.ones((HPC, C * WS, 1), dtype=np.float32)
    va = np.concatenate([vp, ones], axis=2)   # [8, 4224, 65]
    va = va.reshape(HPC, C, WS, D + 1).transpose(0, 2, 1, 3)  # [8, 128, 33, 65]
    va = np.ascontiguousarray(va).astype(np.float16)

    return {"qT": qT, "kT": kT, "v": va}


def kernel(q, k, v):
    q = np.asarray(q, dtype=np.float32)
    k = np.asarray(k, dtype=np.float32)
    v = np.asarray(v, dtype=np.float32)
    qf = q.reshape(B * H, N, D)
    kf = k.reshape(B * H, N, D)
    vf = v.reshape(B * H, N, D)

    nc = _get_nc()
    in_maps = [_prep_core(qf, kf, vf, HPC * c) for c in range(NC)]
    res = run_bass_kernel_spmd(nc, in_maps, core_ids=list(range(NC)))

    outs = []
    for c in range(NC):
        o = res.results[c]["out"].astype(np.float32)  # [4, 128, 32, 130]
        o = o.reshape(PAIRS, 128, W, 2, D + 1)
        o = o[..., :D] / o[..., D:]                   # host-side normalize
        # [pair, q, w, h, d] -> [pair, h, w, q, d] -> [8 heads, 4096, 64]
        o = o.transpose(0, 3, 2, 1, 4).reshape(HPC, N, D)
        outs.append(o)
    return np.concatenate(outs, axis=0).reshape(B, H, N, D).astype(np.float32)


if __name__ == "__main__":
    rng = np.random.default_rng(0)
    q = rng.standard_normal((B, H, N, D), dtype=np.float32)
    k = rng.standard_normal((B, H, N, D), dtype=np.float32)
    v = rng.standard_normal((B, H, N, D), dtype=np.float32)
    o = kernel(q, k, v)
    print("out", o.shape, o.dtype, float(np.abs(o).max()))


# revision 29
# speedup vs baseline: 1.0816x; 1.0090x over previous
"""Local (windowed) attention Trainium2 Bass kernel.

Problem: q,k,v [8, 8, 4096, 64] fp32; window 128, look_backward 1, pad -1.0.
out[b,h,w,i,:] = softmax(scale * q_wi . [k_{w-1}; k_w]) @ [v_{w-1}; v_w]
(with window -1 = all -1.0 pad values, which DO enter the softmax).

Sharding: data-parallel over flat batch*heads (64) -> 8 heads per core.

Per-core layouts (prepared host-side, all fp16 -- fp8 variants measured
over the 2e-2 error gate host-side, so inputs stay 16-bit):
  qT : [4, 128, 4096]  head pair stacked on partitions (d=64 each),
                       free axis = 4096 queries (d-major transposed)
  kT : [4, 128, 4224]  same, one pad chunk (128 keys of -1.0) prepended
                       -> 33 chunks of 128 keys
  v  : [8, 128, 33, 65] per head; partition = key-within-chunk, pad chunk
                       prepended; col 64 = 1.0 (ones col yields softmax l)
  out: [4, 128, 32, 130] UNNORMALIZED out cols for both heads of a pair
                       interleaved per window ([w, h, 65]; col 64 of each
                       head's 65 = denominator l); host divides + unpacks.

Device pipeline per head pair, per key-chunk group (2 chunks):
  MM1 (fp16): scoresT[j, i] per chunk/head; heads alternate PE row-group
              bases 0/64 (run concurrently); psum layout [h0:512|h1:512]
              so each bank sees a single weight base partition (mixing
              bases within a bank hard-crashes the device).
  exp: one full-tile op per group, psum fp32 -> 16-bit P tiles.  Split
       across Act (exact table exp) and DVE (Schraudolph exp2 writing
       bf16 bits via an int16 bitcast) to share the 1 elem/cycle/lane
       elementwise throughput wall; the split ratio is the engine
       balance knob.
  MM2 (16b): out_w[i, 0:65] += P_blockT @ v_aug[p] into merged-head psum
       accumulators [128, 3, 130] (one 2KB bank holds 3 windows x both
       heads), deferred two groups behind MM1 so exp never waits on PE.
  evac: one psum->fp16 copy per 3-window batch covering both heads
       (Act and DVE alternate by a balance set); stores stream out on
       the gpsimd queue in 3 slices per pair.

DMA: kt+qt ride the sync HWDGE queue, v + output stores the gpsimd
SWDGE queue; pair p+1's loads are prefetched during pair p's compute so
the PE/exp engines never see a pair-boundary stall (also keeps the PE's
HAM clock-gate warm).

Accuracy: ~1.1e-2 relative vs the 2e-2 gate (host-simulated and HW
measured): Schraudolph mantissa interpolation (~3%/elem, mostly
cancelled by softmax renormalization) on the DVE share; fp16 operands
contribute ~6e-4.
"""

import os
import sys

for _p in ("/opt/trn_rl_repo", "/opt/pypackages"):
    if os.path.isdir(_p) and _p not in sys.path:
        sys.path.append(_p)

import numpy as np

import concourse.mybir as mybir
import concourse.tile as tile
from concourse import bacc
from concourse.bass_utils import run_bass_kernel_spmd

B, H, N, D = 8, 8, 4096, 64
WS = 128                 # window size
W = N // WS              # 32 windows
C = W + 1                # 33 key chunks incl. pad chunk
NC = 8                   # cores
HPC = (B * H) // NC      # 8 heads per core
PAIRS = HPC // 2         # 4 head pairs per core
SCALE = float(D) ** -0.5

MM1_DT = mybir.dt.float16
MM2_DT = mybir.dt.float16
GROUP = 2                # key chunks per exp batch
EB = 3                   # windows per merged-head psum accumulator bank
                         # (3 * 130 * 4B = 1560 <= 2KB)
NB = (W + EB - 1) // EB  # 11 evac batches per pair (last has 2 windows)

# Schraudolph exp2 offload: for a subset of groups the exp runs on the DVE
# instead of the Act engine, directly in the bf16 bit domain:
#   int16 t = round(s * (128*log2e*SCALE) + (16256 - C))
# t's bits ARE bf16(2^(s*log2e*SCALE)) with linear mantissa interpolation
# (~3% deterministic error); softmax renormalization cancels most of it.
# Exp-engine assignment per pair (group indices exp'd on the DVE, rest
# on Act).  6 of 17 groups (incl. the cheap trimmed last one) on DVE
# plus all evacs balances the two engines at ~12.5us per pair (the
# elementwise throughput wall).
SCH_OFF_BY_PAIR = [
    {2, 5, 8, 11, 14, 16},
    {2, 5, 8, 11, 14, 16},
    {2, 5, 8, 11, 14, 16},
    {2, 5, 8, 11, 14, 16},
]
SCH_A = float(128.0 / np.log(2.0) * SCALE)
SCH_B = float(16256.0 - 5.59)

_NC_CACHE = {}


def build_nc(pairs=PAIRS, w=W):
    c = w + 1
    n = w * WS
    nb = (w + EB - 1) // EB
    nc = bacc.Bacc("TRN2", target_bir_lowering=False)
    qT = nc.dram_tensor("qT", [pairs, 128, n], MM1_DT, kind="ExternalInput")
    kT = nc.dram_tensor("kT", [pairs, 128, c * WS], MM1_DT, kind="ExternalInput")
    vv = nc.dram_tensor("v", [2 * pairs, 128, c, D + 1], MM2_DT, kind="ExternalInput")
    out = nc.dram_tensor("out", [pairs, 128, w, 2 * (D + 1)], mybir.dt.float16,
                         kind="ExternalOutput")

    f32 = mybir.dt.float32
    Exp = mybir.ActivationFunctionType.Exp

    with tile.TileContext(nc) as tc:
        with (
            tc.tile_pool(name="qk", bufs=2) as qk_pool,
            tc.tile_pool(name="vp", bufs=4) as v_pool,
            tc.tile_pool(name="pt", bufs=4) as pt_pool,
            tc.tile_pool(name="st", bufs=2) as st_pool,
            tc.tile_pool(name="ps_s", bufs=3, space="PSUM") as ps_s,
            tc.tile_pool(name="ps_o", bufs=2, space="PSUM") as ps_o,
        ):
            # --- tiles for all pairs, loads for pair 0 up front ---------
            # stg is split into one tile per output store slice so an
            # evac never WAR-waits on a still-in-flight DMA store of an
            # earlier slice (DMA reads hold the whole tile).  The last
            # pair stores batch-by-batch (whole small tiles) so the
            # post-compute drain tail is minimal.
            SLICES_STD = [(0, 12), (12, 21), (21, 30), (30, w)]
            SLICES_LAST = [(0, 12), (12, 21), (21, 24), (24, 27), (27, 30),
                           (30, w)]
            qts, kts, vts, stgs = [], [], [], []
            for pair in range(pairs):
                qts.append(qk_pool.tile([128, n], MM1_DT, tag="qT",
                                        name=f"qt_{pair}"))
                kts.append(qk_pool.tile([128, c * WS], MM1_DT, tag="kT",
                                        name=f"kt_{pair}"))
                vts.append([v_pool.tile([128, c, D + 1], MM2_DT, tag="v",
                                        name=f"v_{pair}_{h}") for h in range(2)])
                slices = SLICES_LAST if pair == pairs - 1 else SLICES_STD
                stgs.append([st_pool.tile([128, e - s, 2 * (D + 1)],
                                          mybir.dt.float16, tag=f"stg{si}",
                                          name=f"stg_{pair}_{si}")
                             for si, (s, e) in enumerate(slices)])

            def load_pair(pair, startup=False):
                qt, kt, vt = qts[pair], kts[pair], vts[pair]
                if startup:
                    # fine-grained need-ordered first slices so MM1 starts
                    # ~1.5us in; qt rides the scalar HWDGE queue (Act is
                    # idle during startup) so kt/qt stream in parallel
                    kb = [0, 256, 1024, 2304, c * WS]
                    qb = [0, 384, 1024, 2304, n]
                    nc.sync.dma_start(kt[:, kb[0]:kb[1]], kT[pair][:, kb[0]:kb[1]])
                    nc.scalar.dma_start(qt[:, qb[0]:qb[1]], qT[pair][:, qb[0]:qb[1]])
                    nc.gpsimd.dma_start(vt[0][:, 0:5], vv[2 * pair][:, 0:5])
                    nc.gpsimd.dma_start(vt[1][:, 0:5], vv[2 * pair + 1][:, 0:5])
                    for sl in (1, 2, 3):
                        nc.sync.dma_start(kt[:, kb[sl]:kb[sl + 1]],
                                          kT[pair][:, kb[sl]:kb[sl + 1]])
                        nc.scalar.dma_start(qt[:, qb[sl]:qb[sl + 1]],
                                            qT[pair][:, qb[sl]:qb[sl + 1]])
                    # keep both heads' v in lockstep with chunk consumption
                    # (a late head-1 slice head-of-line-blocks the PE FIFO
                    # at that head's MM2s)
                    for lo, hi in ((5, 19), (19, c)):
                        nc.gpsimd.dma_start(vt[0][:, lo:hi],
                                            vv[2 * pair][:, lo:hi])
                        nc.gpsimd.dma_start(vt[1][:, lo:hi],
                                            vv[2 * pair + 1][:, lo:hi])

            # prefetch step fn: called at group boundaries of the previous
            # pair; one DMA config each so the queues stay busy end-to-end
            def prefetch_step(pair, step):
                # kt -> sync queue, qt -> scalar queue, v -> gpsimd queue:
                # each queue carries ~1.1MB per pair cycle, well under its
                # bandwidth, so loads always finish before the pair starts
                qt, kt, vt = qts[pair], kts[pair], vts[pair]
                if step == 0:
                    nc.sync.dma_start(kt[:, :], kT[pair][:, :])
                elif step == 1:
                    nc.gpsimd.dma_start(vt[0][:, :], vv[2 * pair][:, :])
                elif step == 2:
                    nc.scalar.dma_start(qt[:, :], qT[pair][:, :])
                elif step == 3:
                    nc.gpsimd.dma_start(vt[1][:, :], vv[2 * pair + 1][:, :])

            # prefetch after pair-0's own (finer) slices have queue priority;
            # HBM bandwidth is the binding resource during the rampup crunch,
            # so the next pair's loads start only once the current pair's
            # tail slices are nearly done
            PREFETCH_AT = {7: 0, 9: 2, 11: 1, 13: 3}  # gi -> prefetch step

            load_pair(0, startup=True)

            # evac batch b (EB windows) -> store slice index
            B2S_STD = [0, 0, 0, 0, 1, 1, 1, 2, 2, 2, 3]
            B2S_LAST = [0, 0, 0, 0, 1, 1, 1, 2, 3, 4, 5]

            for pair in range(pairs):
                qt, kt, vt = qts[pair], kts[pair], vts[pair]
                stg = stgs[pair]
                accum = {}  # batch -> merged psum accumulation tile

                last_pair = pair == pairs - 1
                b2s = B2S_LAST if last_pair else B2S_STD
                slices = SLICES_LAST if last_pair else SLICES_STD

                def emit_evac(b):
                    # evacuate UNNORMALIZED psum (out cols + denominator l
                    # per head) as fp16; the host does out/l
                    nbw = min(EB, w - b * EB)
                    acc = accum.pop(b)
                    si = b2s[b]
                    s0, s1 = slices[si]
                    nc.vector.tensor_copy(
                        stg[si][:, b * EB - s0:b * EB - s0 + nbw],
                        acc[:, 0:nbw, :])
                    if b == nb - 1 or b2s[b + 1] != si:
                        # store the slice once its last batch is evacuated;
                        # the last pair's small final tiles go out on the
                        # idle sync queue
                        seng = nc.sync if (last_pair and si >= 2) else nc.gpsimd
                        seng.dma_start(out[pair][:, s0:s1], stg[si])

                groups = [list(range(g, min(g + GROUP, c)))
                          for g in range(0, c, GROUP)]
                pending_mm2 = []

                def do_mm2s(chunks, pt):
                    # start=True clears has_written for the WHOLE bank, so
                    # with both heads sharing a bank only the batch's very
                    # first matmul may use it; all later writes rely on the
                    # per-element has_written bits (overwrite where clear,
                    # accumulate where set).
                    for s, p in enumerate(chunks):
                        for h in range(2):
                            col = h * (GROUP * 256) + s * 256
                            if p >= 1:
                                # window p-1 self-contribution (stop)
                                wi = p - 1
                                t = accum[wi // EB]
                                nc.tensor.matmul(
                                    t[:, wi % EB, 65 * h:65 * h + 65],
                                    pt[:, col:col + WS],
                                    vt[h][:, p, :],
                                    start=False, stop=True,
                                )
                                if h == 1 and (wi % EB == EB - 1 or wi == w - 1):
                                    emit_evac(wi // EB)
                            if p <= w - 1:
                                # window p prev-contribution
                                bcol = col + (WS if p >= 1 else 0)
                                t = accum.get(p // EB)
                                first = False
                                if t is None:
                                    t = ps_o.tile([128, EB, 2 * (D + 1)], f32,
                                                  tag="out",
                                                  name=f"acc_{pair}_{p // EB}")
                                    accum[p // EB] = t
                                    first = True
                                nc.tensor.matmul(
                                    t[:, p % EB, 65 * h:65 * h + 65],
                                    pt[:, bcol:bcol + WS],
                                    vt[h][:, p, :],
                                    start=first, stop=False,
                                )

                sch_off = SCH_OFF_BY_PAIR[pair % len(SCH_OFF_BY_PAIR)]
                for gi, chunks in enumerate(groups):
                    if pair + 1 < pairs and gi in PREFETCH_AT:
                        prefetch_step(pair + 1, PREFETCH_AT[gi])
                    ps = ps_s.tile([128, GROUP * 2 * 256], f32, tag="scores")
                    # MM1s
                    for s, p in enumerate(chunks):
                        qlo = max(0, (p - 1) * WS)
                        qhi = min(n, (p + 1) * WS)
                        if p == 0:
                            qhi = min(n, 2 * WS)  # avoid garbage: fill 256
                        nq = qhi - qlo
                        for h in range(2):
                            col = h * (GROUP * 256) + s * 256
                            nc.tensor.matmul(
                                ps[:, col:col + nq],
                                kt[64 * h:64 * h + 64, p * WS:(p + 1) * WS],
                                qt[64 * h:64 * h + 64, qlo:qhi],
                                start=True, stop=True,
                            )
                    # one full-tile exp; garbage cols (last chunk's upper
                    # half) are exp'd but never consumed by MM2.  The last
                    # (single-chunk) group only feeds MM2 from cols 0:128
                    # and 512:640 - skip the trailing garbage.
                    ncols = 640 if len(chunks) == 1 else GROUP * 2 * 256
                    if gi in sch_off:
                        pt = pt_pool.tile([128, GROUP * 2 * 256],
                                          mybir.dt.bfloat16, tag="pt")
                        nc.vector.tensor_scalar(
                            pt.bitcast(mybir.dt.int16)[:, 0:ncols],
                            ps[:, 0:ncols], SCH_A, SCH_B,
                            mybir.AluOpType.mult, mybir.AluOpType.add)
                    else:
                        pt = pt_pool.tile([128, GROUP * 2 * 256], MM2_DT,
                                          tag="pt")
                        nc.scalar.activation(pt[:, 0:ncols], ps[:, 0:ncols],
                                             Exp, scale=SCALE)
                    # MM2s deferred two groups: keeps MM1(g+1) ahead of the
                    # Act/DVE exp so the exp engines never wait on the PE.
                    # Shallower near the end so the drain tail is short.
                    pending_mm2.append((chunks, pt))
                    depth = 2 if gi < len(groups) - 2 else 1
                    if len(pending_mm2) > depth:
                        do_mm2s(*pending_mm2.pop(0))
                while pending_mm2:
                    do_mm2s(*pending_mm2.pop(0))

    nc.compile()
    return nc


def _get_nc():
    if "nc" not in _NC_CACHE:
        _NC_CACHE["nc"] = build_nc()
    return _NC_CACHE["nc"]


def _prep_core(qf, kf, vf, lo):
    """Build one core's input dict from flat [64, 4096, 64] fp32 arrays."""
    q8 = qf[lo:lo + HPC]                      # [8, 4096, 64]
    k8 = kf[lo:lo + HPC]
    v8 = vf[lo:lo + HPC]

    qT = np.ascontiguousarray(q8.transpose(0, 2, 1)).reshape(PAIRS, 128, N)
    qT = qT.astype(np.float16)

    pad = np.full((HPC, WS, D), -1.0, dtype=np.float32)
    kp = np.concatenate([pad, k8], axis=1)    # [8, 4224, 64]
    kT = np.ascontiguousarray(kp.transpose(0, 2, 1)).reshape(PAIRS, 128, C * WS)
    kT = kT.astype(np.float16)

    vp = np.concatenate([pad, v8], axis=1)    # [8, 4224, 64]
    ones = np.ones((HPC, C * WS, 1), dtype=np.float32)
    va = np.concatenate([vp, ones], axis=2)   # [8, 4224, 65]
    va = va.reshape(HPC, C, WS, D + 1).transpose(0, 2, 1, 3)  # [8, 128, 33, 65]
    va = np.ascontiguousarray(va).astype(np.float16)

    return {"qT": qT, "kT": kT, "v": va}


def kernel(q, k, v):
    q = np.asarray(q, dtype=np.float32)
    k = np.asarray(k, dtype=np.float32)
    v = np.asarray(v, dtype=np.float32)
    qf = q.reshape(B * H, N, D)
    kf = k.reshape(B * H, N, D)
    vf = v.reshape(B * H, N, D)

    nc = _get_nc()
    in_maps = [_prep_core(qf, kf, vf, HPC * c) for c in range(NC)]
    res = run_bass_kernel_spmd(nc, in_maps, core_ids=list(range(NC)))

    outs = []
    for c in range(NC):
        o = res.results[c]["out"].astype(np.float32)  # [4, 128, 32, 130]
        o = o.reshape(PAIRS, 128, W, 2, D + 1)
        o = o[..., :D] / o[..., D:]                   # host-side normalize
        # [pair, q, w, h, d] -> [pair, h, w, q, d] -> [8 heads, 4096, 64]
        o = o.transpose(0, 3, 2, 1, 4).reshape(HPC, N, D)
        outs.append(o)
    return np.concatenate(outs, axis=0).reshape(B, H, N, D).astype(np.float32)


if __name__ == "__main__":
    rng = np.random.default_rng(0)
    q = rng.standard_normal((B, H, N, D), dtype=np.float32)
    k = rng.standard_normal((B, H, N, D), dtype=np.float32)
    v = rng.standard_normal((B, H, N, D), dtype=np.float32)
    o = kernel(q, k, v)
    print("out", o.shape, o.dtype, float(np.abs(o).max()))


# revision 30
# speedup vs baseline: 1.0945x; 1.0119x over previous
"""Local (windowed) attention Trainium2 Bass kernel.

Problem: q,k,v [8, 8, 4096, 64] fp32; window 128, look_backward 1, pad -1.0.
out[b,h,w,i,:] = softmax(scale * q_wi . [k_{w-1}; k_w]) @ [v_{w-1}; v_w]
(with window -1 = all -1.0 pad values, which DO enter the softmax).

Sharding: data-parallel over flat batch*heads (64) -> 8 heads per core.

Per-core layouts (prepared host-side, all fp16 -- fp8 variants measured
over the 2e-2 error gate host-side, so inputs stay 16-bit):
  qT : [4, 128, 4096]  head pair stacked on partitions (d=64 each),
                       free axis = 4096 queries (d-major transposed)
  kT : [4, 128, 4224]  same, one pad chunk (128 keys of -1.0) prepended
                       -> 33 chunks of 128 keys
  v  : [8, 128, 33, 65] per head; partition = key-within-chunk, pad chunk
                       prepended; col 64 = 1.0 (ones col yields softmax l)
  out: [4, 128, 32, 130] UNNORMALIZED out cols for both heads of a pair
                       interleaved per window ([w, h, 65]; col 64 of each
                       head's 65 = denominator l); host divides + unpacks.

Device pipeline per head pair, per key-chunk group (2 chunks):
  MM1 (fp16): scoresT[j, i] per chunk/head; heads alternate PE row-group
              bases 0/64; psum layout [h0:512|h1:512] so each bank sees a
              single weight base partition (mixing bases within a bank
              hard-crashes the device).
  exp: one full-tile op per group, psum fp32 -> 16-bit P tiles.  Split
       across Act (exact table exp, 11 groups) and DVE (Schraudolph exp2
       writing bf16 bits via an int16 bitcast, 6 groups + all evacs) to
       share the 1 elem/cycle/lane elementwise throughput wall.
  MM2 (16b): out_w[i, 0:65] += P_blockT @ v_aug[p] into merged-head psum
       accumulators [128, 3, 130] (one 2KB bank holds 3 windows x both
       heads; only a batch's FIRST matmul may use start=True - it clears
       has_written for the whole bank - later writes rely on the
       per-element bits), deferred two groups behind MM1 so the exp
       engines never wait on the PE.
  evac: one DVE psum->fp16 copy per 3-window batch covering both heads,
       into per-store-slice stg tiles (so an evac never WAR-waits on an
       in-flight store of an earlier slice).

Steady state is three-way balanced: PE ~13.0us, DVE ~12.9us, Act
~12.6us busy per 14us pair period (MM1's rhs stream is shared by the
head pair at 1 col/cycle, so the PE floor is set by the 2x4096-col
score stream per head).

DMA: kt+qt ride the sync HWDGE queue, qt prefetches the scalar HWDGE
queue, v + mid-kernel stores the gpsimd SWDGE queue; pair 0 loads in
fine need-ordered slices (both heads' v in lockstep - a late head-1
slice head-of-line-blocks the PE FIFO), pair p+1's loads are prefetched
late in pair p (gi 7..13) because aggregate HBM bandwidth is the
binding resource during the rampup crunch.  The last pair stores
batch-by-batch on the idle sync queue to shorten the drain tail.

Accuracy: ~1e-2 relative vs the 2e-2 gate (host-simulated and HW
measured): Schraudolph mantissa interpolation (~3%/elem, mostly
cancelled by softmax renormalization) on the DVE share; fp16 operands
contribute ~6e-4.
"""

import os
import sys

for _p in ("/opt/trn_rl_repo", "/opt/pypackages"):
    if os.path.isdir(_p) and _p not in sys.path:
        sys.path.append(_p)

import numpy as np

import concourse.mybir as mybir
import concourse.tile as tile
from concourse import bacc
from concourse.bass_utils import run_bass_kernel_spmd

B, H, N, D = 8, 8, 4096, 64
WS = 128                 # window size
W = N // WS              # 32 windows
C = W + 1                # 33 key chunks incl. pad chunk
NC = 8                   # cores
HPC = (B * H) // NC      # 8 heads per core
PAIRS = HPC // 2         # 4 head pairs per core
SCALE = float(D) ** -0.5

MM1_DT = mybir.dt.float16
MM2_DT = mybir.dt.float16
GROUP = 2                # key chunks per exp batch
EB = 3                   # windows per merged-head psum accumulator bank
                         # (3 * 130 * 4B = 1560 <= 2KB)
NB = (W + EB - 1) // EB  # 11 evac batches per pair (last has 2 windows)

# Schraudolph exp2 offload: for a subset of groups the exp runs on the DVE
# instead of the Act engine, directly in the bf16 bit domain:
#   int16 t = round(s * (128*log2e*SCALE) + (16256 - C))
# t's bits ARE bf16(2^(s*log2e*SCALE)) with linear mantissa interpolation
# (~3% deterministic error); softmax renormalization cancels most of it.
# Exp-engine assignment per pair (group indices exp'd on the DVE, rest
# on Act).  6 of 17 groups (incl. the cheap trimmed last one) on DVE
# plus all evacs balances the two engines at ~12.5us per pair (the
# elementwise throughput wall).
SCH_OFF_BY_PAIR = [
    {2, 5, 8, 11, 14, 16},
    {2, 5, 8, 11, 14, 16},
    {2, 5, 8, 11, 14, 16},
    {2, 5, 8, 11, 14, 16},
]
SCH_A = float(128.0 / np.log(2.0) * SCALE)
SCH_B = float(16256.0 - 5.59)

_NC_CACHE = {}


def build_nc(pairs=PAIRS, w=W):
    c = w + 1
    n = w * WS
    nb = (w + EB - 1) // EB
    nc = bacc.Bacc("TRN2", target_bir_lowering=False)
    qT = nc.dram_tensor("qT", [pairs, 128, n], MM1_DT, kind="ExternalInput")
    kT = nc.dram_tensor("kT", [pairs, 128, c * WS], MM1_DT, kind="ExternalInput")
    vv = nc.dram_tensor("v", [2 * pairs, 128, c, D + 1], MM2_DT, kind="ExternalInput")
    out = nc.dram_tensor("out", [pairs, 128, w, 2 * (D + 1)], mybir.dt.float16,
                         kind="ExternalOutput")

    f32 = mybir.dt.float32
    Exp = mybir.ActivationFunctionType.Exp

    with tile.TileContext(nc) as tc:
        with (
            tc.tile_pool(name="qk", bufs=2) as qk_pool,
            tc.tile_pool(name="vp", bufs=4) as v_pool,
            tc.tile_pool(name="pt", bufs=4) as pt_pool,
            tc.tile_pool(name="st", bufs=2) as st_pool,
            tc.tile_pool(name="ps_s", bufs=3, space="PSUM") as ps_s,
            tc.tile_pool(name="ps_o", bufs=2, space="PSUM") as ps_o,
        ):
            # --- tiles for all pairs, loads for pair 0 up front ---------
            # stg is split into one tile per output store slice so an
            # evac never WAR-waits on a still-in-flight DMA store of an
            # earlier slice (DMA reads hold the whole tile).  The last
            # pair stores batch-by-batch (whole small tiles) so the
            # post-compute drain tail is minimal.
            SLICES_STD = [(0, 12), (12, 21), (21, 30), (30, w)]
            SLICES_LAST = [(0, 12), (12, 21), (21, 24), (24, 27), (27, 30),
                           (30, w)]
            qts, kts, vts, stgs = [], [], [], []
            for pair in range(pairs):
                qts.append(qk_pool.tile([128, n], MM1_DT, tag="qT",
                                        name=f"qt_{pair}"))
                kts.append(qk_pool.tile([128, c * WS], MM1_DT, tag="kT",
                                        name=f"kt_{pair}"))
                vts.append([v_pool.tile([128, c, D + 1], MM2_DT, tag="v",
                                        name=f"v_{pair}_{h}") for h in range(2)])
                slices = SLICES_LAST if pair == pairs - 1 else SLICES_STD
                stgs.append([st_pool.tile([128, e - s, 2 * (D + 1)],
                                          mybir.dt.float16, tag=f"stg{si}",
                                          name=f"stg_{pair}_{si}")
                             for si, (s, e) in enumerate(slices)])

            def load_pair(pair, startup=False):
                qt, kt, vt = qts[pair], kts[pair], vts[pair]
                if startup:
                    # fine-grained need-ordered first slices so MM1 starts
                    # ~1.5us in; qt rides the scalar HWDGE queue (Act is
                    # idle during startup) so kt/qt stream in parallel
                    kb = [0, 256, 1024, 2304, c * WS]
                    qb = [0, 384, 1024, 2304, n]
                    nc.sync.dma_start(kt[:, kb[0]:kb[1]], kT[pair][:, kb[0]:kb[1]])
                    nc.scalar.dma_start(qt[:, qb[0]:qb[1]], qT[pair][:, qb[0]:qb[1]])
                    nc.gpsimd.dma_start(vt[0][:, 0:5], vv[2 * pair][:, 0:5])
                    nc.gpsimd.dma_start(vt[1][:, 0:5], vv[2 * pair + 1][:, 0:5])
                    for sl in (1, 2, 3):
                        nc.sync.dma_start(kt[:, kb[sl]:kb[sl + 1]],
                                          kT[pair][:, kb[sl]:kb[sl + 1]])
                        nc.scalar.dma_start(qt[:, qb[sl]:qb[sl + 1]],
                                            qT[pair][:, qb[sl]:qb[sl + 1]])
                    # keep both heads' v in lockstep with chunk consumption
                    # (a late head-1 slice head-of-line-blocks the PE FIFO
                    # at that head's MM2s)
                    for lo, hi in ((5, 19), (19, c)):
                        nc.gpsimd.dma_start(vt[0][:, lo:hi],
                                            vv[2 * pair][:, lo:hi])
                        nc.gpsimd.dma_start(vt[1][:, lo:hi],
                                            vv[2 * pair + 1][:, lo:hi])

            # prefetch step fn: called at group boundaries of the previous
            # pair; one DMA config each so the queues stay busy end-to-end
            def prefetch_step(pair, step):
                # kt -> sync queue, qt -> scalar queue, v -> gpsimd queue:
                # each queue carries ~1.1MB per pair cycle, well under its
                # bandwidth, so loads always finish before the pair starts
                qt, kt, vt = qts[pair], kts[pair], vts[pair]
                if step == 0:
                    nc.sync.dma_start(kt[:, :], kT[pair][:, :])
                elif step == 1:
                    nc.gpsimd.dma_start(vt[0][:, :], vv[2 * pair][:, :])
                elif step == 2:
                    nc.scalar.dma_start(qt[:, :], qT[pair][:, :])
                elif step == 3:
                    nc.gpsimd.dma_start(vt[1][:, :], vv[2 * pair + 1][:, :])

            # prefetch after pair-0's own (finer) slices have queue priority;
            # HBM bandwidth is the binding resource during the rampup crunch,
            # so the next pair's loads start only once the current pair's
            # tail slices are nearly done
            PREFETCH_AT = {7: 0, 9: 2, 11: 1, 13: 3}  # gi -> prefetch step

            load_pair(0, startup=True)

            # evac batch b (EB windows) -> store slice index
            B2S_STD = [0, 0, 0, 0, 1, 1, 1, 2, 2, 2, 3]
            B2S_LAST = [0, 0, 0, 0, 1, 1, 1, 2, 3, 4, 5]

            for pair in range(pairs):
                qt, kt, vt = qts[pair], kts[pair], vts[pair]
                stg = stgs[pair]
                accum = {}  # batch -> merged psum accumulation tile

                last_pair = pair == pairs - 1
                b2s = B2S_LAST if last_pair else B2S_STD
                slices = SLICES_LAST if last_pair else SLICES_STD

                def emit_evac(b):
                    # evacuate UNNORMALIZED psum (out cols + denominator l
                    # per head) as fp16; the host does out/l
                    nbw = min(EB, w - b * EB)
                    acc = accum.pop(b)
                    si = b2s[b]
                    s0, s1 = slices[si]
                    nc.vector.tensor_copy(
                        stg[si][:, b * EB - s0:b * EB - s0 + nbw],
                        acc[:, 0:nbw, :])
                    if b == nb - 1 or b2s[b + 1] != si:
                        # store the slice once its last batch is evacuated;
                        # the last pair's small final tiles go out on the
                        # idle sync queue
                        seng = nc.sync if (last_pair and si >= 2) else nc.gpsimd
                        seng.dma_start(out[pair][:, s0:s1], stg[si])

                groups = [list(range(g, min(g + GROUP, c)))
                          for g in range(0, c, GROUP)]
                pending_mm2 = []

                def do_mm2s(chunks, pt):
                    # start=True clears has_written for the WHOLE bank, so
                    # with both heads sharing a bank only the batch's very
                    # first matmul may use it; all later writes rely on the
                    # per-element has_written bits (overwrite where clear,
                    # accumulate where set).
                    for s, p in enumerate(chunks):
                        for h in range(2):
                            col = h * (GROUP * 256) + s * 256
                            if p >= 1:
                                # window p-1 self-contribution (stop)
                                wi = p - 1
                                t = accum[wi // EB]
                                nc.tensor.matmul(
                                    t[:, wi % EB, 65 * h:65 * h + 65],
                                    pt[:, col:col + WS],
                                    vt[h][:, p, :],
                                    start=False, stop=True,
                                )
                                if h == 1 and (wi % EB == EB - 1 or wi == w - 1):
                                    emit_evac(wi // EB)
                            if p <= w - 1:
                                # window p prev-contribution
                                bcol = col + (WS if p >= 1 else 0)
                                t = accum.get(p // EB)
                                first = False
                                if t is None:
                                    t = ps_o.tile([128, EB, 2 * (D + 1)], f32,
                                                  tag="out",
                                                  name=f"acc_{pair}_{p // EB}")
                                    accum[p // EB] = t
                                    first = True
                                nc.tensor.matmul(
                                    t[:, p % EB, 65 * h:65 * h + 65],
                                    pt[:, bcol:bcol + WS],
                                    vt[h][:, p, :],
                                    start=first, stop=False,
                                )

                sch_off = SCH_OFF_BY_PAIR[pair % len(SCH_OFF_BY_PAIR)]
                for gi, chunks in enumerate(groups):
                    if pair + 1 < pairs and gi in PREFETCH_AT:
                        prefetch_step(pair + 1, PREFETCH_AT[gi])
                    ps = ps_s.tile([128, GROUP * 2 * 256], f32, tag="scores")
                    # MM1s
                    for s, p in enumerate(chunks):
                        qlo = max(0, (p - 1) * WS)
                        qhi = min(n, (p + 1) * WS)
                        if p == 0:
                            qhi = min(n, 2 * WS)  # avoid garbage: fill 256
                        nq = qhi - qlo
                        for h in range(2):
                            col = h * (GROUP * 256) + s * 256
                            nc.tensor.matmul(
                                ps[:, col:col + nq],
                                kt[64 * h:64 * h + 64, p * WS:(p + 1) * WS],
                                qt[64 * h:64 * h + 64, qlo:qhi],
                                start=True, stop=True,
                            )
                    # one full-tile exp; garbage cols (last chunk's upper
                    # half) are exp'd but never consumed by MM2.  The last
                    # (single-chunk) group only feeds MM2 from cols 0:128
                    # and 512:640 - skip the trailing garbage.
                    ncols = 640 if len(chunks) == 1 else GROUP * 2 * 256
                    if gi in sch_off:
                        pt = pt_pool.tile([128, GROUP * 2 * 256],
                                          mybir.dt.bfloat16, tag="pt")
                        nc.vector.tensor_scalar(
                            pt.bitcast(mybir.dt.int16)[:, 0:ncols],
                            ps[:, 0:ncols], SCH_A, SCH_B,
                            mybir.AluOpType.mult, mybir.AluOpType.add)
                    else:
                        pt = pt_pool.tile([128, GROUP * 2 * 256], MM2_DT,
                                          tag="pt")
                        nc.scalar.activation(pt[:, 0:ncols], ps[:, 0:ncols],
                                             Exp, scale=SCALE)
                    # MM2s deferred two groups: keeps MM1(g+1) ahead of the
                    # Act/DVE exp so the exp engines never wait on the PE.
                    # Shallower near the end so the drain tail is short.
                    pending_mm2.append((chunks, pt))
                    depth = 2 if gi < len(groups) - 2 else 1
                    if len(pending_mm2) > depth:
                        do_mm2s(*pending_mm2.pop(0))
                while pending_mm2:
                    do_mm2s(*pending_mm2.pop(0))

    nc.compile()
    return nc


def _get_nc():
    if "nc" not in _NC_CACHE:
        _NC_CACHE["nc"] = build_nc()
    return _NC_CACHE["nc"]


def _prep_core(qf, kf, vf, lo):
    """Build one core's input dict from flat [64, 4096, 64] fp32 arrays."""
    q8 = qf[lo:lo + HPC]                      # [8, 4096, 64]
    k8 = kf[lo:lo + HPC]
    v8 = vf[lo:lo + HPC]

    qT = np.ascontiguousarray(q8.transpose(0, 2, 1)).reshape(PAIRS, 128, N)
    qT = qT.astype(np.float16)

    pad = np.full((HPC, WS, D), -1.0, dtype=np.float32)
    kp = np.concatenate([pad, k8], axis=1)    # [8, 4224, 64]
    kT = np.ascontiguousarray(kp.transpose(0, 2, 1)).reshape(PAIRS, 128, C * WS)
    kT = kT.astype(np.float16)

    vp = np.concatenate([pad, v8], axis=1)    # [8, 4224, 64]
    ones = np.ones((HPC, C * WS, 1), dtype=np.float32)
    va = np.concatenate([vp, ones], axis=2)   # [8, 4224, 65]
    va = va.reshape(HPC, C, WS, D + 1).transpose(0, 2, 1, 3)  # [8, 128, 33, 65]
    va = np.ascontiguousarray(va).astype(np.float16)

    return {"qT": qT, "kT": kT, "v": va}


def kernel(q, k, v):
    q = np.asarray(q, dtype=np.float32)
    k = np.asarray(k, dtype=np.float32)
    v = np.asarray(v, dtype=np.float32)
    qf = q.reshape(B * H, N, D)
    kf = k.reshape(B * H, N, D)
    vf = v.reshape(B * H, N, D)

    nc = _get_nc()
    in_maps = [_prep_core(qf, kf, vf, HPC * c) for c in range(NC)]
    res = run_bass_kernel_spmd(nc, in_maps, core_ids=list(range(NC)))

    outs = []
    for c in range(NC):
        o = res.results[c]["out"].astype(np.float32)  # [4, 128, 32, 130]
        o = o.reshape(PAIRS, 128, W, 2, D + 1)
        o = o[..., :D] / o[..., D:]                   # host-side normalize
        # [pair, q, w, h, d] -> [pair, h, w, q, d] -> [8 heads, 4096, 64]
        o = o.transpose(0, 3, 2, 1, 4).reshape(HPC, N, D)
        outs.append(o)
    return np.concatenate(outs, axis=0).reshape(B, H, N, D).astype(np.float32)


if __name__ == "__main__":
    rng = np.random.default_rng(0)
    q = rng.standard_normal((B, H, N, D), dtype=np.float32)
    k = rng.standard_normal((B, H, N, D), dtype=np.float32)
    v = rng.standard_normal((B, H, N, D), dtype=np.float32)
    o = kernel(q, k, v)
    print("out", o.shape, o.dtype, float(np.abs(o).max()))


# revision 31
# speedup vs baseline: 1.0974x; 1.0026x over previous
"""Local (windowed) attention Trainium2 Bass kernel.

Problem: q,k,v [8, 8, 4096, 64] fp32; window 128, look_backward 1, pad -1.0.
out[b,h,w,i,:] = softmax(scale * q_wi . [k_{w-1}; k_w]) @ [v_{w-1}; v_w]
(with window -1 = all -1.0 pad values, which DO enter the softmax).

Sharding: data-parallel over flat batch*heads (64) -> 8 heads per core.

Per-core layouts (prepared host-side, all fp16 -- fp8 variants measured
over the 2e-2 error gate host-side, so inputs stay 16-bit):
  qT : [4, 128, 4096]  head pair stacked on partitions (d=64 each),
                       free axis = 4096 queries (d-major transposed)
  kT : [4, 128, 4224]  same, one pad chunk (128 keys of -1.0) prepended
                       -> 33 chunks of 128 keys
  v  : [8, 128, 33, 65] per head; partition = key-within-chunk, pad chunk
                       prepended; col 64 = 1.0 (ones col yields softmax l)
  out: [4, 128, 32, 130] UNNORMALIZED out cols for both heads of a pair
                       interleaved per window ([w, h, 65]; col 64 of each
                       head's 65 = denominator l); host divides + unpacks.

Device pipeline per head pair, per key-chunk group (2 chunks):
  MM1 (fp16): scoresT[j, i] per chunk/head; heads alternate PE row-group
              bases 0/64; psum layout [h0:512|h1:512] so each bank sees a
              single weight base partition (mixing bases within a bank
              hard-crashes the device).
  exp: one full-tile op per group, psum fp32 -> 16-bit P tiles.  Split
       across Act (exact table exp, 11 groups) and DVE (Schraudolph exp2
       writing bf16 bits via an int16 bitcast, 6 groups + all evacs) to
       share the 1 elem/cycle/lane elementwise throughput wall.
  MM2 (16b): out_w[i, 0:65] += P_blockT @ v_aug[p] into merged-head psum
       accumulators [128, 3, 130] (one 2KB bank holds 3 windows x both
       heads; only a batch's FIRST matmul may use start=True - it clears
       has_written for the whole bank - later writes rely on the
       per-element bits), deferred two groups behind MM1 so the exp
       engines never wait on the PE.
  evac: one DVE psum->fp16 copy per 3-window batch covering both heads,
       into per-store-slice stg tiles (so an evac never WAR-waits on an
       in-flight store of an earlier slice).

Steady state is three-way balanced: PE ~13.0us, DVE ~12.9us, Act
~12.6us busy per 14us pair period (MM1's rhs stream is shared by the
head pair at 1 col/cycle, so the PE floor is set by the 2x4096-col
score stream per head).

DMA: kt+qt ride the sync HWDGE queue, qt prefetches the scalar HWDGE
queue, v + mid-kernel stores the gpsimd SWDGE queue; pair 0 loads in
fine need-ordered slices (both heads' v in lockstep - a late head-1
slice head-of-line-blocks the PE FIFO), pair p+1's loads are prefetched
late in pair p (gi 7..13) because aggregate HBM bandwidth is the
binding resource during the rampup crunch.  The last pair stores
batch-by-batch on the idle sync queue to shorten the drain tail.

Accuracy: ~1e-2 relative vs the 2e-2 gate (host-simulated and HW
measured): Schraudolph mantissa interpolation (~3%/elem, mostly
cancelled by softmax renormalization) on the DVE share; fp16 operands
contribute ~6e-4.
"""

import os
import sys

for _p in ("/opt/trn_rl_repo", "/opt/pypackages"):
    if os.path.isdir(_p) and _p not in sys.path:
        sys.path.append(_p)

import numpy as np

import concourse.mybir as mybir
import concourse.tile as tile
from concourse import bacc
from concourse.bass_utils import run_bass_kernel_spmd

B, H, N, D = 8, 8, 4096, 64
WS = 128                 # window size
W = N // WS              # 32 windows
C = W + 1                # 33 key chunks incl. pad chunk
NC = 8                   # cores
HPC = (B * H) // NC      # 8 heads per core
PAIRS = HPC // 2         # 4 head pairs per core
SCALE = float(D) ** -0.5

MM1_DT = mybir.dt.float16
MM2_DT = mybir.dt.float16
GROUP = 2                # key chunks per exp batch
EB = 3                   # windows per merged-head psum accumulator bank
                         # (3 * 130 * 4B = 1560 <= 2KB)
NB = (W + EB - 1) // EB  # 11 evac batches per pair (last has 2 windows)

# Schraudolph exp2 offload: for a subset of groups the exp runs on the DVE
# instead of the Act engine, directly in the bf16 bit domain:
#   int16 t = round(s * (128*log2e*SCALE) + (16256 - C))
# t's bits ARE bf16(2^(s*log2e*SCALE)) with linear mantissa interpolation
# (~3% deterministic error); softmax renormalization cancels most of it.
# Exp-engine assignment per pair (group indices exp'd on the DVE, rest
# on Act).  6 of 17 groups (incl. the cheap trimmed last one) on DVE
# plus all evacs balances the two engines at ~12.5us per pair (the
# elementwise throughput wall).
SCH_OFF_BY_PAIR = [
    {2, 5, 8, 11, 14, 16},
    {2, 5, 8, 11, 14, 16},
    {2, 5, 8, 11, 14, 16},
    {2, 5, 8, 11, 14, 16},
]
SCH_A = float(128.0 / np.log(2.0) * SCALE)
SCH_B = float(16256.0 - 5.59)

_NC_CACHE = {}


def build_nc(pairs=PAIRS, w=W):
    c = w + 1
    n = w * WS
    nb = (w + EB - 1) // EB
    nc = bacc.Bacc("TRN2", target_bir_lowering=False)
    qT = nc.dram_tensor("qT", [pairs, 128, n], MM1_DT, kind="ExternalInput")
    kT = nc.dram_tensor("kT", [pairs, 128, c * WS], MM1_DT, kind="ExternalInput")
    vv = nc.dram_tensor("v", [2 * pairs, 128, c, D + 1], MM2_DT, kind="ExternalInput")
    out = nc.dram_tensor("out", [pairs, 128, w, 2 * (D + 1)], mybir.dt.float16,
                         kind="ExternalOutput")

    f32 = mybir.dt.float32
    Exp = mybir.ActivationFunctionType.Exp

    with tile.TileContext(nc) as tc:
        with (
            tc.tile_pool(name="qk", bufs=2) as qk_pool,
            tc.tile_pool(name="vp", bufs=4) as v_pool,
            tc.tile_pool(name="pt", bufs=4) as pt_pool,
            tc.tile_pool(name="st", bufs=2) as st_pool,
            tc.tile_pool(name="ps_s", bufs=3, space="PSUM") as ps_s,
            tc.tile_pool(name="ps_o", bufs=2, space="PSUM") as ps_o,
        ):
            # --- tiles for all pairs, loads for pair 0 up front ---------
            # stg is split into one tile per output store slice so an
            # evac never WAR-waits on a still-in-flight DMA store of an
            # earlier slice (DMA reads hold the whole tile).  The last
            # pair stores batch-by-batch (whole small tiles) so the
            # post-compute drain tail is minimal.
            SLICES_STD = [(0, 12), (12, 21), (21, 30), (30, w)]
            SLICES_LAST = [(0, 12), (12, 21), (21, 24), (24, 27), (27, 30),
                           (30, w)]
            qts, kts, vts, stgs = [], [], [], []
            for pair in range(pairs):
                qts.append(qk_pool.tile([128, n], MM1_DT, tag="qT",
                                        name=f"qt_{pair}"))
                kts.append(qk_pool.tile([128, c * WS], MM1_DT, tag="kT",
                                        name=f"kt_{pair}"))
                vts.append([v_pool.tile([128, c, D + 1], MM2_DT, tag="v",
                                        name=f"v_{pair}_{h}") for h in range(2)])
                slices = SLICES_LAST if pair == pairs - 1 else SLICES_STD
                stgs.append([st_pool.tile([128, e - s, 2 * (D + 1)],
                                          mybir.dt.float16, tag=f"stg{si}",
                                          name=f"stg_{pair}_{si}")
                             for si, (s, e) in enumerate(slices)])

            def load_pair(pair, startup=False):
                qt, kt, vt = qts[pair], kts[pair], vts[pair]
                if startup:
                    # fine-grained need-ordered first slices so MM1 starts
                    # ~1.5us in; qt rides the scalar HWDGE queue (Act is
                    # idle during startup) so kt/qt stream in parallel
                    kb = [0, 256, 1024, 2304, c * WS]
                    qb = [0, 384, 1024, 2304, n]
                    nc.sync.dma_start(kt[:, kb[0]:kb[1]], kT[pair][:, kb[0]:kb[1]])
                    nc.scalar.dma_start(qt[:, qb[0]:qb[1]], qT[pair][:, qb[0]:qb[1]])
                    nc.gpsimd.dma_start(vt[0][:, 0:5], vv[2 * pair][:, 0:5])
                    nc.gpsimd.dma_start(vt[1][:, 0:5], vv[2 * pair + 1][:, 0:5])
                    for sl in (1, 2, 3):
                        nc.sync.dma_start(kt[:, kb[sl]:kb[sl + 1]],
                                          kT[pair][:, kb[sl]:kb[sl + 1]])
                        nc.scalar.dma_start(qt[:, qb[sl]:qb[sl + 1]],
                                            qT[pair][:, qb[sl]:qb[sl + 1]])
                    # keep both heads' v in lockstep with chunk consumption
                    # (a late head-1 slice head-of-line-blocks the PE FIFO
                    # at that head's MM2s)
                    for lo, hi in ((5, 19), (19, c)):
                        nc.gpsimd.dma_start(vt[0][:, lo:hi],
                                            vv[2 * pair][:, lo:hi])
                        nc.gpsimd.dma_start(vt[1][:, lo:hi],
                                            vv[2 * pair + 1][:, lo:hi])

            # prefetch step fn: called at group boundaries of the previous
            # pair; one DMA config each so the queues stay busy end-to-end
            def prefetch_step(pair, step):
                # kt -> sync queue, qt -> scalar queue, v -> gpsimd queue:
                # each queue carries ~1.1MB per pair cycle, well under its
                # bandwidth, so loads always finish before the pair starts
                qt, kt, vt = qts[pair], kts[pair], vts[pair]
                if step == 0:
                    nc.sync.dma_start(kt[:, :], kT[pair][:, :])
                elif step == 1:
                    nc.gpsimd.dma_start(vt[0][:, :], vv[2 * pair][:, :])
                elif step == 2:
                    nc.scalar.dma_start(qt[:, :], qT[pair][:, :])
                elif step == 3:
                    nc.gpsimd.dma_start(vt[1][:, :], vv[2 * pair + 1][:, :])

            # prefetch after pair-0's own (finer) slices have queue priority;
            # HBM bandwidth is the binding resource during the rampup crunch,
            # so the next pair's loads start only once the current pair's
            # tail slices are nearly done
            PREFETCH_AT = {7: 0, 9: 2, 11: 1, 13: 3}  # gi -> prefetch step

            load_pair(0, startup=True)

            # evac batch b (EB windows) -> store slice index
            B2S_STD = [0, 0, 0, 0, 1, 1, 1, 2, 2, 2, 3]
            B2S_LAST = [0, 0, 0, 0, 1, 1, 1, 2, 3, 4, 5]

            for pair in range(pairs):
                qt, kt, vt = qts[pair], kts[pair], vts[pair]
                stg = stgs[pair]
                accum = {}  # batch -> merged psum accumulation tile

                last_pair = pair == pairs - 1
                b2s = B2S_LAST if last_pair else B2S_STD
                slices = SLICES_LAST if last_pair else SLICES_STD

                def emit_evac(b):
                    # evacuate UNNORMALIZED psum (out cols + denominator l
                    # per head) as fp16; the host does out/l
                    nbw = min(EB, w - b * EB)
                    acc = accum.pop(b)
                    si = b2s[b]
                    s0, s1 = slices[si]
                    nc.vector.tensor_copy(
                        stg[si][:, b * EB - s0:b * EB - s0 + nbw],
                        acc[:, 0:nbw, :])
                    if b == nb - 1 or b2s[b + 1] != si:
                        # store the slice once its last batch is evacuated;
                        # the last pair's small final tiles go out on the
                        # idle sync queue
                        seng = nc.sync if (last_pair and si >= 2) else nc.gpsimd
                        seng.dma_start(out[pair][:, s0:s1], stg[si])

                groups = [list(range(g, min(g + GROUP, c)))
                          for g in range(0, c, GROUP)]
                pending_mm2 = []

                def do_mm2s(chunks, pt):
                    # start=True clears has_written for the WHOLE bank, so
                    # with both heads sharing a bank only the batch's very
                    # first matmul may use it; all later writes rely on the
                    # per-element has_written bits (overwrite where clear,
                    # accumulate where set).
                    for s, p in enumerate(chunks):
                        for h in range(2):
                            col = h * (GROUP * 256) + s * 256
                            if p >= 1:
                                # window p-1 self-contribution (stop)
                                wi = p - 1
                                t = accum[wi // EB]
                                nc.tensor.matmul(
                                    t[:, wi % EB, 65 * h:65 * h + 65],
                                    pt[:, col:col + WS],
                                    vt[h][:, p, :],
                                    start=False, stop=True,
                                )
                                if h == 1 and (wi % EB == EB - 1 or wi == w - 1):
                                    emit_evac(wi // EB)
                            if p <= w - 1:
                                # window p prev-contribution
                                bcol = col + (WS if p >= 1 else 0)
                                t = accum.get(p // EB)
                                first = False
                                if t is None:
                                    t = ps_o.tile([128, EB, 2 * (D + 1)], f32,
                                                  tag="out",
                                                  name=f"acc_{pair}_{p // EB}")
                                    accum[p // EB] = t
                                    first = True
                                nc.tensor.matmul(
                                    t[:, p % EB, 65 * h:65 * h + 65],
                                    pt[:, bcol:bcol + WS],
                                    vt[h][:, p, :],
                                    start=first, stop=False,
                                )

                sch_off = SCH_OFF_BY_PAIR[pair % len(SCH_OFF_BY_PAIR)]

                def emit_mm1(gix):
                    ps = ps_s.tile([128, GROUP * 2 * 256], f32, tag="scores")
                    for s, p in enumerate(groups[gix]):
                        qlo = max(0, (p - 1) * WS)
                        qhi = min(n, (p + 1) * WS)
                        if p == 0:
                            qhi = min(n, 2 * WS)  # avoid garbage: fill 256
                        nq = qhi - qlo
                        for h in range(2):
                            col = h * (GROUP * 256) + s * 256
                            nc.tensor.matmul(
                                ps[:, col:col + nq],
                                kt[64 * h:64 * h + 64, p * WS:(p + 1) * WS],
                                qt[64 * h:64 * h + 64, qlo:qhi],
                                start=True, stop=True,
                            )
                    return ps

                # MM1s run TWO groups ahead of the exp (ps_s bufs=3 allows
                # exactly 3 score tiles in flight) so the exp engines never
                # wait on the PE FIFO reaching their group's MM1s
                ps_q = [emit_mm1(0), emit_mm1(1)]
                for gi, chunks in enumerate(groups):
                    if pair + 1 < pairs and gi in PREFETCH_AT:
                        prefetch_step(pair + 1, PREFETCH_AT[gi])
                    if gi + 2 < len(groups):
                        ps_q.append(emit_mm1(gi + 2))
                    ps = ps_q.pop(0)
                    # one full-tile exp; garbage cols (last chunk's upper
                    # half) are exp'd but never consumed by MM2.  The last
                    # (single-chunk) group only feeds MM2 from cols 0:128
                    # and 512:640 - skip the trailing garbage.
                    ncols = 640 if len(chunks) == 1 else GROUP * 2 * 256
                    if gi in sch_off:
                        pt = pt_pool.tile([128, GROUP * 2 * 256],
                                          mybir.dt.bfloat16, tag="pt")
                        nc.vector.tensor_scalar(
                            pt.bitcast(mybir.dt.int16)[:, 0:ncols],
                            ps[:, 0:ncols], SCH_A, SCH_B,
                            mybir.AluOpType.mult, mybir.AluOpType.add)
                    else:
                        pt = pt_pool.tile([128, GROUP * 2 * 256], MM2_DT,
                                          tag="pt")
                        nc.scalar.activation(pt[:, 0:ncols], ps[:, 0:ncols],
                                             Exp, scale=SCALE)
                    # MM2s deferred two groups: keeps MM1(g+1) ahead of the
                    # Act/DVE exp so the exp engines never wait on the PE.
                    # Shallower near the end so the drain tail is short.
                    pending_mm2.append((chunks, pt))
                    depth = 2 if gi < len(groups) - 2 else 1
                    if len(pending_mm2) > depth:
                        do_mm2s(*pending_mm2.pop(0))
                while pending_mm2:
                    do_mm2s(*pending_mm2.pop(0))

    nc.compile()
    return nc


def _get_nc():
    if "nc" not in _NC_CACHE:
        _NC_CACHE["nc"] = build_nc()
    return _NC_CACHE["nc"]


def _prep_core(qf, kf, vf, lo):
    """Build one core's input dict from flat [64, 4096, 64] fp32 arrays."""
    q8 = qf[lo:lo + HPC]                      # [8, 4096, 64]
    k8 = kf[lo:lo + HPC]
    v8 = vf[lo:lo + HPC]

    qT = np.ascontiguousarray(q8.transpose(0, 2, 1)).reshape(PAIRS, 128, N)
    qT = qT.astype(np.float16)

    pad = np.full((HPC, WS, D), -1.0, dtype=np.float32)
    kp = np.concatenate([pad, k8], axis=1)    # [8, 4224, 64]
    kT = np.ascontiguousarray(kp.transpose(0, 2, 1)).reshape(PAIRS, 128, C * WS)
    kT = kT.astype(np.float16)

    vp = np.concatenate([pad, v8], axis=1)    # [8, 4224, 64]
    ones = np.ones((HPC, C * WS, 1), dtype=np.float32)
    va = np.concatenate([vp, ones], axis=2)   # [8, 4224, 65]
    va = va.reshape(HPC, C, WS, D + 1).transpose(0, 2, 1, 3)  # [8, 128, 33, 65]
    va = np.ascontiguousarray(va).astype(np.float16)

    return {"qT": qT, "kT": kT, "v": va}


def kernel(q, k, v):
    q = np.asarray(q, dtype=np.float32)
    k = np.asarray(k, dtype=np.float32)
    v = np.asarray(v, dtype=np.float32)
    qf = q.reshape(B * H, N, D)
    kf = k.reshape(B * H, N, D)
    vf = v.reshape(B * H, N, D)

    nc = _get_nc()
    in_maps = [_prep_core(qf, kf, vf, HPC * c) for c in range(NC)]
    res = run_bass_kernel_spmd(nc, in_maps, core_ids=list(range(NC)))

    outs = []
    for c in range(NC):
        o = res.results[c]["out"].astype(np.float32)  # [4, 128, 32, 130]
        o = o.reshape(PAIRS, 128, W, 2, D + 1)
        o = o[..., :D] / o[..., D:]                   # host-side normalize
        # [pair, q, w, h, d] -> [pair, h, w, q, d] -> [8 heads, 4096, 64]
        o = o.transpose(0, 3, 2, 1, 4).reshape(HPC, N, D)
        outs.append(o)
    return np.concatenate(outs, axis=0).reshape(B, H, N, D).astype(np.float32)


if __name__ == "__main__":
    rng = np.random.default_rng(0)
    q = rng.standard_normal((B, H, N, D), dtype=np.float32)
    k = rng.standard_normal((B, H, N, D), dtype=np.float32)
    v = rng.standard_normal((B, H, N, D), dtype=np.float32)
    o = kernel(q, k, v)
    print("out", o.shape, o.dtype, float(np.abs(o).max()))
